# revision 17
# baseline (speedup 1.0000x reference)
"""BSpline KAN layer (grid_size=5, spline_order=3) on 8 Trainium2 NeuronCores.

Strategy (data-parallel over batch):
  - Each core gets B_local = 512 rows of x, replicated weights.
  - Layout on-chip: in-dim on partitions (8 chunks of 128), batch on free dim.

Fast path (host-detected uniform identical grid, which setup_inputs produces):
  With u = (x - g0)/h the cubic bases are the cardinal B-spline b3_j = S(u-j).
  Using the two-sided truncated-power form with m_j = 2 - |u - (j+2)|:
      b3_j = (1/6)*relu(m_j)^3 - (2/3)*relu(m_j - 1)^3
  (exact: S is symmetric about its center; for m <= 2 the remaining truncated
  terms vanish, and all values are bounded by 8 so fp16 is safe).
  Per in-chunk: 8 narrow ACT Abs ops (immediate scale/bias) + 1 ACT Square +
  7 wide DVE ops produce the full [128, 8, 512] basis stack; the 1/6 is folded
  into the spline weights on the host.  ACT ~10us/chunk, DVE ~11us/chunk, both
  below the tensor engine's ~18us/chunk matmul stream.

  - Spline contraction as matmul with k-order j-major: k = j*1024 + i, so the
    j-stacked basis tiles are directly the matmul rhs. silu(x) @ base_weight.T
    is folded in as a 9th "basis" with base_weight as its weight block.
  - All 8 PSUM banks accumulate the 8 out-chunks across the whole contraction;
    epilogue adds res_scale * x and stores y[out, batch] (host transposes).

Fallback path (general grids): Cox-de Boor recursion kernel (unchanged from
the general implementation; correct for any grid).
Precision: fp16 bases/weights, fp32 accumulation.
"""

import numpy as np

import concourse.bass as bass
from concourse import bacc
import concourse.mybir as mybir
import concourse.tile as tile
from concourse.alu_op_type import AluOpType
from concourse.bass_utils import run_bass_kernel_spmd

F32 = mybir.dt.float32
F16 = mybir.dt.float16
AF = mybir.ActivationFunctionType

IN_DIM = 1024
OUT_DIM = 1024
BATCH = 4096
N_CORES = 8
BL = BATCH // N_CORES        # 512 batch rows per core
NCH = IN_DIM // 128          # 8 in-dim chunks
NK = 12                      # knots per dim
EPS = 1e-8

LAST_PROFILE = {}


def _build_nc_uniform(inv_h, nu0):
    """Uniform-grid kernel.  u = x*inv_h + nu0 (nu0 = -g0/h);
    A_j = |u - (j+2)|, m_j = 2 - A_j,
    B3_j = 6*b3_j = relu(m_j)^3 - 4*relu(m_j-1)^3  (weights pre-divided by 6).
    Computed as: PT = min(A-2,0) = -relu(m);  QT = min(A-1,0) = -relu(m-1);
    SP = PT^2*PT = -relu(m)^3 ; SQ = -relu(m-1)^3 ; B3 = 4*SQ - SP.

    Matmuls run bases-stationary / weights-moving: lhsT = B3[in128, batch128]
    so each stationary serves both 512-wide out-halves (halves LDWEIGHTS).
    PSUM bank (bt, oh) = [128 batch, 512 out]; y is emitted [batch, out]."""
    nc = bacc.Bacc("TRN2", target_bir_lowering=False)

    xt = nc.dram_tensor("xt", [IN_DIM, BL], F32, kind="ExternalInput")
    xb = nc.dram_tensor("xb", [BL, IN_DIM], F32, kind="ExternalInput")
    w = nc.dram_tensor("w", [9 * IN_DIM, OUT_DIM], F16, kind="ExternalInput")
    rs = nc.dram_tensor("rs", [1, 1], F32, kind="ExternalInput")
    y = nc.dram_tensor("y", [BL, OUT_DIM], F32, kind="ExternalOutput")

    NBT = BL // 128              # 4 batch tiles
    NOH = OUT_DIM // 512         # 2 out halves

    with tile.TileContext(nc) as tc:
        with (
            tc.tile_pool(name="const", bufs=1) as cp,
            tc.tile_pool(name="xres", bufs=1) as xp,
            tc.tile_pool(name="sil", bufs=2) as sp,
            tc.tile_pool(name="abs", bufs=2) as apl,
            tc.tile_pool(name="cube", bufs=2) as bp,
            tc.tile_pool(name="b3", bufs=2) as b3p,
            tc.tile_pool(name="wts", bufs=3) as wp,
            tc.tile_pool(name="yout", bufs=4) as yp,
            tc.tile_pool(name="psum", bufs=1, space="PSUM") as pp,
        ):
            rs_t = cp.tile([128, 1], F32)
            nc.gpsimd.dma_start(out=rs_t[:, :], in_=rs[:].to_broadcast((128, 1)))

            # per-j ACT biases nu0 - (j+2) as [128,1] columns (no const-AP
            # registration for arbitrary floats); memset on the idle DVE so
            # they are ready before the first Abs
            bias_t = cp.tile([128, 8], F32)
            for j in range(8):
                nc.vector.memset(bias_t[:, j:j + 1], float(nu0 - (j + 2)))

            # dummy Silu: triggers the one-time load of silu_and_others (the
            # only table set needed: it contains abs/square/silu/copy) while
            # the first x tile is still streaming in
            warm = cp.tile([128, 2], F32)
            nc.vector.memset(warm[:, 0:1], 0.0)
            nc.scalar.activation(warm[:, 1:2], warm[:, 0:1], AF.Silu)

            # PSUM accumulators: bank (bt, oh) = [128 batch, 512 out]
            psum = [pp.tile([128, 512], F32, tag=f"ps{b}", name=f"ps{b}")
                    for b in range(NBT * NOH)]

            HB = 4 * BL          # half-stack width (j 0-3 | 4-7)
            xb_tiles = []

            for c in range(NCH):
                # x on the GPSIMD SWDGE queue: parallel to the weight stream
                xc = xp.tile([128, BL], F32, tag=f"xc{c}")
                nc.gpsimd.dma_start(out=xc[:, :],
                                    in_=xt[c * 128:(c + 1) * 128, :])

                # flat [128, 8*BL] stacks: 1-D free dim so wide DVE ops pay the
                # SBUF inter-instruction bubble once, not per 512-row
                A = apl.tile([128, 8 * BL], F16, tag="A")
                sil = sp.tile([128, BL], F16, tag="sil")
                if c == 0:
                    # silu first: the j=8 matmul block starts the tensor
                    # engine while the Abs/cube chain is still filling
                    nc.scalar.activation(sil[:, :], xc[:, :], AF.Silu)
                for j in range(8):
                    nc.scalar.activation(A[:, j * BL:(j + 1) * BL], xc[:, :],
                                         AF.Abs, bias=bias_t[:, j:j + 1],
                                         scale=float(inv_h))
                if c > 0:
                    nc.scalar.activation(sil[:, :], xc[:, :], AF.Silu)

                PT = bp.tile([128, 8 * BL], F16, tag="PT")
                QT = bp.tile([128, 8 * BL], F16, tag="QT")
                SP = bp.tile([128, 8 * BL], F16, tag="SP")
                SQ = bp.tile([128, 8 * BL], F16, tag="SQ")
                B3 = b3p.tile([128, 8 * BL], F16, tag="B3")

                # per slice s: PT = -relu(m), QT = -relu(m-1) (4x tensor_scalar)
                # SQ = -relu(m-1)^3 via 2 DVE mults; SP = relu(m)^2 (ACT Square
                # in steady state, DVE for the latency-critical first chunk),
                # cubed on DVE; B3 = 4*SQ - SP = 6*b3.
                def basis_ops(s, dve_square):
                    nc.vector.tensor_scalar(PT[:, s], A[:, s], 2.0, 0.0,
                                            AluOpType.subtract, AluOpType.min)
                    nc.vector.tensor_scalar(QT[:, s], A[:, s], 1.0, 0.0,
                                            AluOpType.subtract, AluOpType.min)
                    nc.vector.tensor_tensor(SQ[:, s], QT[:, s], QT[:, s],
                                            AluOpType.mult)
                    nc.vector.tensor_tensor(SQ[:, s], SQ[:, s], QT[:, s],
                                            AluOpType.mult)
                    if dve_square:
                        nc.vector.tensor_tensor(SP[:, s], PT[:, s], PT[:, s],
                                                AluOpType.mult)
                    else:
                        nc.scalar.activation(SP[:, s], PT[:, s], AF.Square)
                    nc.vector.tensor_tensor(SP[:, s], SP[:, s], PT[:, s],
                                            AluOpType.mult)
                    nc.vector.scalar_tensor_tensor(B3[:, s], SQ[:, s], 4.0,
                                                   SP[:, s],
                                                   AluOpType.mult,
                                                   AluOpType.subtract)

                if c == 0:
                    # quarter-split, all-DVE: shortest path to the first bases
                    for q in range(4):
                        basis_ops(slice(q * 2 * BL, (q + 1) * 2 * BL), True)
                else:
                    for h in range(2):
                        basis_ops(slice(h * HB, (h + 1) * HB), False)

                # weight blocks (8 spline j's + silu/base_weight), moving
                # operand.  Host layout is chunk-major: rows (c*9+j)*128+p, so
                # a whole chunk is one contiguous DMA; queues alternate
                # sync/scalar to double weight bandwidth.  Early chunks split
                # finer so the first-consumed blocks land first.
                JORDER = ([8, 0, 1, 2, 3, 4, 5, 6, 7] if c == 0
                          else [0, 1, 2, 3, 8, 4, 5, 6, 7])
                qeng = nc.sync if c % 2 == 0 else nc.scalar
                wt = wp.tile([128, 9 * OUT_DIM], F16, tag="wt", name=f"wt{c}")
                base = c * 9 * 128
                if c < 2:
                    jgroups = ([(8, 9), (0, 4), (4, 8)] if c == 0
                               else [(0, 4), (8, 9), (4, 8)])
                else:
                    jgroups = [(0, 9)]
                for j0, j1 in jgroups:
                    qeng.dma_start(
                        out=wt[:, j0 * OUT_DIM:j1 * OUT_DIM]
                        .rearrange("p (j o) -> p j o", o=OUT_DIM),
                        in_=w[base + j0 * 128:base + j1 * 128, :]
                        .rearrange("(j p) o -> p j o", p=128))

                if c == NCH - 1:
                    # batch-major x tiles for the residual epilogue (only
                    # needed now; keeps early DMA bandwidth for weights)
                    for bt in range(NBT):
                        xbt = cp.tile([128, IN_DIM], F32, tag=f"xb{bt}")
                        nc.gpsimd.dma_start(out=xbt[:, :],
                                            in_=xb[bt * 128:(bt + 1) * 128, :])
                        xb_tiles.append(xbt)

                def stat_of(j, bt):
                    if j < 8:
                        return B3[:, j * BL + bt * 128:j * BL + (bt + 1) * 128]
                    return sil[:, bt * 128:(bt + 1) * 128]

                def rhs_of(j, oh):
                    return wt[:, j * OUT_DIM + oh * 512:
                              j * OUT_DIM + (oh + 1) * 512]

                # consume in production order
                if c < NCH - 1:
                    for j in JORDER:
                        for bt in range(NBT):
                            for oh in range(NOH):
                                nc.tensor.matmul(
                                    psum[bt * NOH + oh][:, :],
                                    lhsT=stat_of(j, bt),
                                    rhs=rhs_of(j, oh),
                                    start=(c == 0 and j == 8),
                                    stop=False,
                                    skip_group_check=True)
                else:
                    # last chunk: bt-outer so each PSUM bank pair finishes
                    # early and its epilogue overlaps the remaining matmuls
                    for bt in range(NBT):
                        for j in JORDER:
                            for oh in range(NOH):
                                nc.tensor.matmul(
                                    psum[bt * NOH + oh][:, :],
                                    lhsT=stat_of(j, bt),
                                    rhs=rhs_of(j, oh),
                                    start=False,
                                    stop=(j == 7),
                                    skip_group_check=True)
                        for oh in range(NOH):
                            # ScalarE drains PSUM (it sits closest to PSUM),
                            # DVE adds the residual all-SBUF
                            yt = yp.tile([128, 512], F32, tag="yt",
                                         name=f"yt{bt}_{oh}")
                            nc.scalar.activation(yt[:, :],
                                                 psum[bt * NOH + oh][:, :],
                                                 AF.Copy)
                            nc.vector.scalar_tensor_tensor(
                                yt[:, :],
                                xb_tiles[bt][:, oh * 512:(oh + 1) * 512],
                                rs_t[:, :], yt[:, :],
                                AluOpType.mult, AluOpType.add)
                            nc.gpsimd.dma_start(
                                out=y[bt * 128:(bt + 1) * 128,
                                      oh * 512:(oh + 1) * 512],
                                in_=yt[:, :])

    nc.compile()
    return nc


def _build_nc_general():
    """General-grid fallback: Cox-de Boor recursion on device."""
    nc = bacc.Bacc("TRN2", target_bir_lowering=False)

    xt = nc.dram_tensor("xt", [IN_DIM, BL], F32, kind="ExternalInput")
    w = nc.dram_tensor("w", [9 * IN_DIM, OUT_DIM], F16, kind="ExternalInput")
    gsl = nc.dram_tensor("gsl", [128, NCH * (NK - 1)], F32, kind="ExternalInput")
    gst = nc.dram_tensor("gst", [128, NCH], F32, kind="ExternalInput")
    rs = nc.dram_tensor("rs", [1, 1], F32, kind="ExternalInput")
    y = nc.dram_tensor("y", [OUT_DIM, BL], F32, kind="ExternalOutput")

    with tile.TileContext(nc) as tc:
        with (
            tc.tile_pool(name="const", bufs=1) as cp,
            tc.tile_pool(name="xres", bufs=1) as xp,
            tc.tile_pool(name="small", bufs=4) as sp,
            tc.tile_pool(name="updn", bufs=2) as bp1,
            tc.tile_pool(name="lr2", bufs=2) as bp2,
            tc.tile_pool(name="lr3", bufs=3) as bp3,
            tc.tile_pool(name="wts", bufs=12) as wp,
            tc.tile_pool(name="yout", bufs=4) as yp,
            tc.tile_pool(name="psum", bufs=1, space="PSUM") as pp,
        ):
            # ---------------- grid preparation (once) ----------------
            gslT = cp.tile([128, NK - 1, NCH], F32)
            nc.gpsimd.dma_start(out=gslT[:, :, :],
                                in_=gsl[:, :].rearrange("p (k c) -> p k c", c=NCH))
            g3 = cp.tile([128, NK, NCH], F32)
            nc.gpsimd.dma_start(out=g3[:, 0, :], in_=gst[:, :])

            # softplus(v) = relu(v) + ln(1 + exp(-|v|))
            st3 = cp.tile([128, NK - 1, NCH], F32)
            spa = cp.tile([128, NK - 1, NCH], F32)
            nc.scalar.activation(spa[:, :, :], gslT[:, :, :], AF.Abs)
            nc.scalar.activation(spa[:, :, :], spa[:, :, :], AF.Exp, scale=-1.0)
            nc.scalar.activation(spa[:, :, :], spa[:, :, :], AF.Ln, bias=1.0)
            nc.scalar.activation(st3[:, :, :], gslT[:, :, :], AF.Relu)
            nc.vector.tensor_tensor(st3[:, :, :], st3[:, :, :], spa[:, :, :],
                                    AluOpType.add)
            for k in range(1, NK):
                nc.vector.tensor_tensor(g3[:, k, :], g3[:, k - 1, :],
                                        st3[:, k - 1, :], AluOpType.add)

            def recips(d, n):
                dt = cp.tile([128, n, NCH], F32, tag=f"d{d}")
                nc.vector.tensor_tensor(dt[:, :, :], g3[:, d:NK, :],
                                        g3[:, 0:NK - d, :], AluOpType.subtract)
                nc.vector.tensor_scalar_add(dt[:, :, :], dt[:, :, :], EPS)
                r = cp.tile([128, n, NCH], F32, tag=f"r{d}")
                nc.vector.reciprocal(r[:, :, :], dt[:, :, :])
                nr = cp.tile([128, n, NCH], F32, tag=f"nr{d}")
                nc.vector.tensor_scalar_mul(nr[:, :, :], r[:, :, :], -1.0)
                return r, nr

            R1, NR1 = recips(1, NK - 1)   # [128,8,11]
            R2, NR2 = recips(2, NK - 2)   # [128,8,10]
            R3, NR3 = recips(3, NK - 3)   # [128,8,9]

            # biases for the ACT hat ops
            BU = cp.tile([128, 10, NCH], F32)   # -g[j]*R1[j]
            nc.vector.scalar_tensor_tensor(BU[:, :, :], g3[:, 0:10, :], -1.0,
                                           R1[:, 0:10, :],
                                           AluOpType.mult, AluOpType.mult)
            BD = cp.tile([128, 10, NCH], F32)   # g[j+2]*R1[j+1]
            nc.vector.tensor_tensor(BD[:, :, :], g3[:, 2:12, :],
                                    R1[:, 1:11, :], AluOpType.mult)

            # biases for the ACT degree-3 factor ops
            BL3 = cp.tile([128, 8, NCH], F32)   # -g[j]*R3[j]
            nc.vector.scalar_tensor_tensor(BL3[:, :, :], g3[:, 0:8, :], -1.0,
                                           R3[:, 0:8, :],
                                           AluOpType.mult, AluOpType.mult)
            BR3 = cp.tile([128, 8, NCH], F32)   # g[j+4]*R3[j+1]
            nc.vector.tensor_tensor(BR3[:, :, :], g3[:, 4:12, :],
                                    R3[:, 1:9, :], AluOpType.mult)

            rs_t = cp.tile([128, 1], F32)
            nc.gpsimd.dma_start(out=rs_t[:, :], in_=rs[:].to_broadcast((128, 1)))

            psum = [pp.tile([128, BL], F32, tag=f"ps{m}", name=f"ps{m}")
                    for m in range(NCH)]

            xc_tiles = []
            for c in range(NCH):
                xc = xp.tile([128, BL], F32, tag=f"xc{c}")
                nc.sync.dma_start(out=xc[:, :], in_=xt[c * 128:(c + 1) * 128, :])
                xc_tiles.append(xc)

                x16 = sp.tile([128, BL], F16, tag="x16")
                nc.vector.tensor_scalar(x16[:, :], xc[:, :], 1.0,
                                        None, AluOpType.mult)
                UP = bp1.tile([128, 10, BL], F16, tag="up")
                DN = bp1.tile([128, 10, BL], F16, tag="dn")
                for j in range(10):
                    nc.scalar.activation(UP[:, j, :], x16[:, :], AF.Relu,
                                         bias=BU[:, j, c:c+1], scale=R1[:, j, c:c+1])
                    nc.scalar.activation(DN[:, j, :], x16[:, :], AF.Relu,
                                         bias=BD[:, j, c:c+1], scale=NR1[:, j+1, c:c+1])

                L2 = bp2.tile([128, 10, BL], F16, tag="l2")
                R2t = bp2.tile([128, 9, BL], F16, tag="r2")
                L3 = bp3.tile([128, 8, BL], F16, tag="l3")
                R3t = bp3.tile([128, 8, BL], F16, tag="r3")
                for j in range(10):
                    nc.vector.tensor_scalar(L2[:, j, :], x16[:, :],
                                            g3[:, j, c:c+1], R2[:, j, c:c+1],
                                            AluOpType.subtract, AluOpType.mult)
                nc.vector.tensor_scalar(R2t[:, :, :], L2[:, 1:10, :], -1.0,
                                        1.0, AluOpType.mult, AluOpType.add)
                for j in range(3):
                    nc.vector.tensor_scalar(L3[:, j, :], x16[:, :],
                                            g3[:, j, c:c+1], R3[:, j, c:c+1],
                                            AluOpType.subtract, AluOpType.mult)
                for j in range(8):
                    if j >= 3:
                        nc.scalar.activation(L3[:, j, :], x16[:, :], AF.Identity,
                                             bias=BL3[:, j, c:c+1],
                                             scale=R3[:, j, c:c+1])
                    nc.scalar.activation(R3t[:, j, :], x16[:, :], AF.Identity,
                                         bias=BR3[:, j, c:c+1],
                                         scale=NR3[:, j+1, c:c+1])
                sil = sp.tile([128, BL], F16, tag="sil")
                nc.scalar.activation(sil[:, :], x16[:, :], AF.Silu)

                nc.vector.tensor_tensor(UP[:, :, :], UP[:, :, :], DN[:, :, :],
                                        AluOpType.min)   # b1 := UP
                nc.vector.tensor_tensor(L2[:, 0:9, :], L2[:, 0:9, :],
                                        UP[:, 0:9, :], AluOpType.mult)
                nc.vector.tensor_tensor(R2t[:, :, :], R2t[:, :, :],
                                        UP[:, 1:10, :], AluOpType.mult)
                nc.vector.tensor_tensor(L2[:, 0:9, :], L2[:, 0:9, :],
                                        R2t[:, :, :], AluOpType.add)  # b2
                nc.vector.tensor_tensor(L3[:, :, :], L3[:, :, :],
                                        L2[:, 0:8, :], AluOpType.mult)
                nc.vector.tensor_tensor(R3t[:, :, :], R3t[:, :, :],
                                        L2[:, 1:9, :], AluOpType.mult)
                nc.vector.tensor_tensor(L3[:, :, :], L3[:, :, :],
                                        R3t[:, :, :], AluOpType.add)  # b3

                wts = []
                for j in range(9):
                    kc = j * NCH + c
                    wt = wp.tile([128, OUT_DIM], F16, tag="wt", name=f"wt{c}_{j}")
                    nc.sync.dma_start(out=wt[:, :],
                                      in_=w[kc * 128:(kc + 1) * 128, :])
                    wts.append(wt)

                def rhs_of(j):
                    return L3[:, j, :] if j < 8 else sil[:, :]

                if c < NCH - 1:
                    for j in range(9):
                        for m in range(NCH):
                            nc.tensor.matmul(psum[m][:, :],
                                             lhsT=wts[j][:, m * 128:(m + 1) * 128],
                                             rhs=rhs_of(j),
                                             start=(c == 0 and j == 0),
                                             stop=False,
                                             skip_group_check=True)
                else:
                    for m in range(NCH):
                        for j in range(9):
                            nc.tensor.matmul(psum[m][:, :],
                                             lhsT=wts[j][:, m * 128:(m + 1) * 128],
                                             rhs=rhs_of(j),
                                             start=False,
                                             stop=(j == 8),
                                             skip_group_check=True)
                        yt = yp.tile([128, BL], F32, tag="yt", name=f"yt{m}")
                        nc.vector.scalar_tensor_tensor(yt[:, :],
                                                       xc_tiles[m][:, :],
                                                       rs_t[:, :], psum[m][:, :],
                                                       AluOpType.mult,
                                                       AluOpType.add)
                        nc.sync.dma_start(out=y[m * 128:(m + 1) * 128, :],
                                          in_=yt[:, :])

    nc.compile()
    return nc


_NC_CACHE = {}


def kernel(x, coeffs, base_weight, grid_steps_log, grid_start, res_scale,
           _trace=False):
    global LAST_PROFILE

    x = np.asarray(x, dtype=np.float32)
    coeffs = np.asarray(coeffs, dtype=np.float32)
    base_weight = np.asarray(base_weight, dtype=np.float32)
    grid_steps_log = np.asarray(grid_steps_log, dtype=np.float32)
    grid_start = np.asarray(grid_start, dtype=np.float32)
    res_scale = np.asarray(res_scale, dtype=np.float32)

    # ---- host-side grid analysis (float64) ----
    steps64 = np.logaddexp(0.0, grid_steps_log.astype(np.float64))  # softplus
    g0_64 = grid_start.astype(np.float64)[:, 0]
    h = float(steps64.mean())
    g0 = float(g0_64.mean())
    uniform = (np.abs(steps64 - h).max() <= 1e-6 * max(abs(h), 1e-12)
               and np.abs(g0_64 - g0).max() <= 1e-6 and h > 0)

    xT = np.ascontiguousarray(x.T)                                # [in, B]
    rs_r = res_scale.reshape(1, 1)

    # weight blocks; block j=8 is base_weight.T
    wj = coeffs.reshape(OUT_DIM, IN_DIM, 8).transpose(2, 1, 0)    # [8, in, out]
    if uniform:
        wj = wj * (1.0 / 6.0)        # fold the 1/6 of the cardinal spline
    big_w = np.concatenate([wj, base_weight.T[None]], axis=0)     # [9, in, out]
    if uniform:
        # chunk-major row order: row (c*9 + j)*128 + p  (one DMA per chunk)
        big_w = (big_w.reshape(9, NCH, 128, OUT_DIM).transpose(1, 0, 2, 3)
                 .reshape(9 * IN_DIM, OUT_DIM))
    else:
        # j-major row order: k = j*IN_DIM + i
        big_w = big_w.reshape(9 * IN_DIM, OUT_DIM)
    big_w = np.ascontiguousarray(big_w, dtype=np.float16)

    if uniform:
        key = ("uniform", round(1.0 / h, 9), round(-g0 / h, 9))
        if key not in _NC_CACHE:
            _NC_CACHE.clear()
            _NC_CACHE[key] = _build_nc_uniform(1.0 / h, -g0 / h)
        nc = _NC_CACHE[key]
        in_maps = [{
            "xt": np.ascontiguousarray(xT[:, c * BL:(c + 1) * BL]),
            "xb": np.ascontiguousarray(x[c * BL:(c + 1) * BL, :]),
            "w": big_w,
            "rs": rs_r,
        } for c in range(N_CORES)]
    else:
        key = ("general",)
        if key not in _NC_CACHE:
            _NC_CACHE.clear()
            _NC_CACHE[key] = _build_nc_general()
        nc = _NC_CACHE[key]
        gsl_r = np.ascontiguousarray(
            grid_steps_log.reshape(NCH, 128, NK - 1).transpose(1, 2, 0)
            .reshape(128, (NK - 1) * NCH))
        gst_r = np.ascontiguousarray(grid_start.reshape(NCH, 128).T)
        in_maps = [{
            "xt": np.ascontiguousarray(xT[:, c * BL:(c + 1) * BL]),
            "w": big_w,
            "gsl": gsl_r,
            "gst": gst_r,
            "rs": rs_r,
        } for c in range(N_CORES)]

    res = run_bass_kernel_spmd(nc, in_maps, core_ids=list(range(N_CORES)),
                               trace=_trace)
    LAST_PROFILE = {
        "exec_time_ns": res.exec_time_ns,
        "mean_exec_time_ns": res.mean_exec_time_ns,
        "max_exec_time_core_id": res.max_exec_time_core_id,
        "profile_json": res.profile_json,
        "instructions_and_trace": res.instructions_and_trace,
    }

    if uniform:
        out = np.concatenate([r["y"] for r in res.results], axis=0)  # [B, out]
    else:
        out = np.concatenate([r["y"].T for r in res.results], axis=0)
    return np.ascontiguousarray(out.astype(np.float32))


# revision 22
# speedup vs baseline: 1.0371x; 1.0371x over previous
"""BSpline KAN layer (grid_size=5, spline_order=3) on 8 Trainium2 NeuronCores.

Strategy (data-parallel over batch):
  - Each core gets B_local = 512 rows of x, replicated weights.
  - Layout on-chip: in-dim on partitions (8 chunks of 128), batch on free dim.

Fast path (host-detected uniform identical grid, which setup_inputs produces):
  With u = (x - g0)/h the cubic bases are the cardinal B-spline b3_j = S(u-j).
  Using the two-sided truncated-power form with m_j = 2 - |u - (j+2)|:
      b3_j = (1/6)*relu(m_j)^3 - (2/3)*relu(m_j - 1)^3
  (exact: S is symmetric about its center; for m <= 2 the remaining truncated
  terms vanish, and all values are bounded by 8 so fp16 is safe).
  Per in-chunk: 8 narrow ACT Abs ops (immediate scale/bias) + 1 ACT Square +
  7 wide DVE ops produce the full [128, 8, 512] basis stack; the 1/6 is folded
  into the spline weights on the host.  ACT ~10us/chunk, DVE ~11us/chunk, both
  below the tensor engine's ~18us/chunk matmul stream.

  - Spline contraction as matmul with k-order j-major: k = j*1024 + i, so the
    j-stacked basis tiles are directly the matmul rhs. silu(x) @ base_weight.T
    is folded in as a 9th "basis" with base_weight as its weight block.
  - All 8 PSUM banks accumulate the 8 out-chunks across the whole contraction;
    epilogue adds res_scale * x and stores y[out, batch] (host transposes).

Fallback path (general grids): Cox-de Boor recursion kernel (unchanged from
the general implementation; correct for any grid).
Precision: fp16 bases/weights, fp32 accumulation.
"""

import numpy as np

import concourse.bass as bass
from concourse import bacc
import concourse.mybir as mybir
import concourse.tile as tile
from concourse.alu_op_type import AluOpType
from concourse.bass_utils import run_bass_kernel_spmd

F32 = mybir.dt.float32
F16 = mybir.dt.float16
AF = mybir.ActivationFunctionType

IN_DIM = 1024
OUT_DIM = 1024
BATCH = 4096
N_CORES = 8
BL = BATCH // N_CORES        # 512 batch rows per core
NCH = IN_DIM // 128          # 8 in-dim chunks
NK = 12                      # knots per dim
EPS = 1e-8

LAST_PROFILE = {}


def _build_nc_uniform(inv_h, nu0):
    """Uniform-grid kernel.  u = x*inv_h + nu0 (nu0 = -g0/h);
    A_j = |u - (j+2)|, m_j = 2 - A_j,
    B3_j = 6*b3_j = relu(m_j)^3 - 4*relu(m_j-1)^3  (weights pre-divided by 6).
    Computed as: PT = min(A-2,0) = -relu(m);  QT = min(A-1,0) = -relu(m-1);
    SP = PT^2*PT = -relu(m)^3 ; SQ = -relu(m-1)^3 ; B3 = 4*SQ - SP.

    Matmuls run bases-stationary / weights-moving: lhsT = B3[in128, batch128]
    so each stationary serves both 512-wide out-halves (halves LDWEIGHTS).
    PSUM bank (bt, oh) = [128 batch, 512 out]; y is emitted [batch, out]."""
    nc = bacc.Bacc("TRN2", target_bir_lowering=False)

    xt = nc.dram_tensor("xt", [IN_DIM, BL], F32, kind="ExternalInput")
    xb = nc.dram_tensor("xb", [BL, IN_DIM], F32, kind="ExternalInput")
    w = nc.dram_tensor("w", [9 * IN_DIM, OUT_DIM], F16, kind="ExternalInput")
    rs = nc.dram_tensor("rs", [1, 1], F32, kind="ExternalInput")
    y = nc.dram_tensor("y", [BL, OUT_DIM], F32, kind="ExternalOutput")

    NBT = BL // 128              # 4 batch tiles
    NOH = OUT_DIM // 512         # 2 out halves

    with tile.TileContext(nc) as tc:
        with (
            tc.tile_pool(name="const", bufs=1) as cp,
            tc.tile_pool(name="xres", bufs=1) as xp,
            tc.tile_pool(name="sil", bufs=2) as sp,
            tc.tile_pool(name="abs", bufs=2) as apl,
            tc.tile_pool(name="cube", bufs=2) as bp,
            tc.tile_pool(name="b3", bufs=2) as b3p,
            tc.tile_pool(name="wts", bufs=3) as wp,
            tc.tile_pool(name="yout", bufs=4) as yp,
            tc.tile_pool(name="psum", bufs=1, space="PSUM") as pp,
        ):
            rs_t = cp.tile([128, 1], F32)
            nc.gpsimd.dma_start(out=rs_t[:, :], in_=rs[:].to_broadcast((128, 1)))

            # per-j ACT biases nu0 - (j+2) as [128,1] columns (no const-AP
            # registration for arbitrary floats); memset on the idle DVE so
            # they are ready before the first Abs
            bias_t = cp.tile([128, 8], F32)
            for j in range(8):
                nc.vector.memset(bias_t[:, j:j + 1], float(nu0 - (j + 2)))

            # dummy Silu: triggers the one-time load of silu_and_others (the
            # only table set needed: it contains abs/square/silu/copy) while
            # the first x tile is still streaming in
            warm = cp.tile([128, 2], F32)
            nc.vector.memset(warm[:, 0:1], 0.0)
            nc.scalar.activation(warm[:, 1:2], warm[:, 0:1], AF.Silu)

            # PSUM accumulators: bank (bt, oh) = [128 batch, 512 out]
            psum = [pp.tile([128, 512], F32, tag=f"ps{b}", name=f"ps{b}")
                    for b in range(NBT * NOH)]

            HB = 4 * BL          # half-stack width (j 0-3 | 4-7)
            xb_tiles = []

            for c in range(NCH):
                # x at the head of the scalar HW queue (ahead of the odd
                # weight blocks), parallel to the sync weight stream
                xc = xp.tile([128, BL], F32, tag=f"xc{c}")
                nc.scalar.dma_start(out=xc[:, :],
                                    in_=xt[c * 128:(c + 1) * 128, :])

                # flat [128, 8*BL] stacks: 1-D free dim so wide DVE ops pay the
                # SBUF inter-instruction bubble once, not per 512-row
                A = apl.tile([128, 8 * BL], F16, tag="A")
                sil = sp.tile([128, BL], F16, tag="sil")
                if c == 0:
                    # silu first: the j=8 matmul block starts the tensor
                    # engine while the Abs/cube chain is still filling
                    nc.scalar.activation(sil[:, :], xc[:, :], AF.Silu)
                for j in range(8):
                    nc.scalar.activation(A[:, j * BL:(j + 1) * BL], xc[:, :],
                                         AF.Abs, bias=bias_t[:, j:j + 1],
                                         scale=float(inv_h))
                if c > 0:
                    nc.scalar.activation(sil[:, :], xc[:, :], AF.Silu)

                PT = bp.tile([128, 8 * BL], F16, tag="PT")
                QT = bp.tile([128, 8 * BL], F16, tag="QT")
                SP = bp.tile([128, 8 * BL], F16, tag="SP")
                SQ = bp.tile([128, 8 * BL], F16, tag="SQ")
                B3 = b3p.tile([128, 8 * BL], F16, tag="B3")

                # per slice s: PT = -relu(m), QT = -relu(m-1) (4x tensor_scalar)
                # SQ = -relu(m-1)^3 via 2 DVE mults; SP = relu(m)^2 (ACT Square
                # in steady state, DVE for the latency-critical first chunk),
                # cubed on DVE; B3 = 4*SQ - SP = 6*b3.
                def basis_ops(s, dve_square):
                    nc.vector.tensor_scalar(PT[:, s], A[:, s], 2.0, 0.0,
                                            AluOpType.subtract, AluOpType.min)
                    nc.vector.tensor_scalar(QT[:, s], A[:, s], 1.0, 0.0,
                                            AluOpType.subtract, AluOpType.min)
                    nc.vector.tensor_tensor(SQ[:, s], QT[:, s], QT[:, s],
                                            AluOpType.mult)
                    nc.vector.tensor_tensor(SQ[:, s], SQ[:, s], QT[:, s],
                                            AluOpType.mult)
                    if dve_square:
                        nc.vector.tensor_tensor(SP[:, s], PT[:, s], PT[:, s],
                                                AluOpType.mult)
                    else:
                        nc.scalar.activation(SP[:, s], PT[:, s], AF.Square)
                    nc.vector.tensor_tensor(SP[:, s], SP[:, s], PT[:, s],
                                            AluOpType.mult)
                    nc.vector.scalar_tensor_tensor(B3[:, s], SQ[:, s], 4.0,
                                                   SP[:, s],
                                                   AluOpType.mult,
                                                   AluOpType.subtract)

                if c == 0:
                    # quarter-split, all-DVE: shortest path to the first bases
                    for q in range(4):
                        basis_ops(slice(q * 2 * BL, (q + 1) * 2 * BL), True)
                else:
                    for h in range(2):
                        basis_ops(slice(h * HB, (h + 1) * HB), False)

                # weight blocks (8 spline j's + silu/base_weight), moving
                # operand.  Host layout is chunk-major: rows (c*9+j)*128+p.
                # Per-block DMAs in consumption order, alternating between the
                # sync and scalar HW queues: doubles weight bandwidth while
                # matmuls unblock block-by-block.
                JORDER = ([8, 0, 1, 2, 3, 4, 5, 6, 7] if c == 0
                          else [0, 1, 2, 3, 8, 4, 5, 6, 7])
                wt = wp.tile([128, 9 * OUT_DIM], F16, tag="wt", name=f"wt{c}")
                base = c * 9 * 128
                for idx, j in enumerate(JORDER):
                    qeng = nc.sync if idx % 2 == 0 else nc.scalar
                    qeng.dma_start(
                        out=wt[:, j * OUT_DIM:(j + 1) * OUT_DIM],
                        in_=w[base + j * 128:base + (j + 1) * 128, :])

                if c == NCH - 1:
                    # batch-major x tiles for the residual epilogue (only
                    # needed now; keeps early DMA bandwidth for weights)
                    for bt in range(NBT):
                        xbt = cp.tile([128, IN_DIM], F32, tag=f"xb{bt}")
                        nc.scalar.dma_start(out=xbt[:, :],
                                            in_=xb[bt * 128:(bt + 1) * 128, :])
                        xb_tiles.append(xbt)

                def stat_of(j, bt):
                    if j < 8:
                        return B3[:, j * BL + bt * 128:j * BL + (bt + 1) * 128]
                    return sil[:, bt * 128:(bt + 1) * 128]

                def rhs_of(j, oh):
                    return wt[:, j * OUT_DIM + oh * 512:
                              j * OUT_DIM + (oh + 1) * 512]

                # consume in production order
                if c < NCH - 1:
                    for j in JORDER:
                        for bt in range(NBT):
                            for oh in range(NOH):
                                nc.tensor.matmul(
                                    psum[bt * NOH + oh][:, :],
                                    lhsT=stat_of(j, bt),
                                    rhs=rhs_of(j, oh),
                                    start=(c == 0 and j == 8),
                                    stop=False,
                                    skip_group_check=True)
                else:
                    # last chunk: bt-outer so each PSUM bank pair finishes
                    # early and its epilogue overlaps the remaining matmuls
                    for bt in range(NBT):
                        for j in JORDER:
                            for oh in range(NOH):
                                nc.tensor.matmul(
                                    psum[bt * NOH + oh][:, :],
                                    lhsT=stat_of(j, bt),
                                    rhs=rhs_of(j, oh),
                                    start=False,
                                    stop=(j == 7),
                                    skip_group_check=True)
                        for oh in range(NOH):
                            # ScalarE drains PSUM (it sits closest to PSUM),
                            # DVE adds the residual all-SBUF
                            yt = yp.tile([128, 512], F32, tag="yt",
                                         name=f"yt{bt}_{oh}")
                            nc.scalar.activation(yt[:, :],
                                                 psum[bt * NOH + oh][:, :],
                                                 AF.Copy)
                            nc.vector.scalar_tensor_tensor(
                                yt[:, :],
                                xb_tiles[bt][:, oh * 512:(oh + 1) * 512],
                                rs_t[:, :], yt[:, :],
                                AluOpType.mult, AluOpType.add)
                            nc.sync.dma_start(
                                out=y[bt * 128:(bt + 1) * 128,
                                      oh * 512:(oh + 1) * 512],
                                in_=yt[:, :])

    nc.compile()
    return nc


def _build_nc_general():
    """General-grid fallback: Cox-de Boor recursion on device."""
    nc = bacc.Bacc("TRN2", target_bir_lowering=False)

    xt = nc.dram_tensor("xt", [IN_DIM, BL], F32, kind="ExternalInput")
    w = nc.dram_tensor("w", [9 * IN_DIM, OUT_DIM], F16, kind="ExternalInput")
    gsl = nc.dram_tensor("gsl", [128, NCH * (NK - 1)], F32, kind="ExternalInput")
    gst = nc.dram_tensor("gst", [128, NCH], F32, kind="ExternalInput")
    rs = nc.dram_tensor("rs", [1, 1], F32, kind="ExternalInput")
    y = nc.dram_tensor("y", [OUT_DIM, BL], F32, kind="ExternalOutput")

    with tile.TileContext(nc) as tc:
        with (
            tc.tile_pool(name="const", bufs=1) as cp,
            tc.tile_pool(name="xres", bufs=1) as xp,
            tc.tile_pool(name="small", bufs=4) as sp,
            tc.tile_pool(name="updn", bufs=2) as bp1,
            tc.tile_pool(name="lr2", bufs=2) as bp2,
            tc.tile_pool(name="lr3", bufs=3) as bp3,
            tc.tile_pool(name="wts", bufs=12) as wp,
            tc.tile_pool(name="yout", bufs=4) as yp,
            tc.tile_pool(name="psum", bufs=1, space="PSUM") as pp,
        ):
            # ---------------- grid preparation (once) ----------------
            gslT = cp.tile([128, NK - 1, NCH], F32)
            nc.gpsimd.dma_start(out=gslT[:, :, :],
                                in_=gsl[:, :].rearrange("p (k c) -> p k c", c=NCH))
            g3 = cp.tile([128, NK, NCH], F32)
            nc.gpsimd.dma_start(out=g3[:, 0, :], in_=gst[:, :])

            # softplus(v) = relu(v) + ln(1 + exp(-|v|))
            st3 = cp.tile([128, NK - 1, NCH], F32)
            spa = cp.tile([128, NK - 1, NCH], F32)
            nc.scalar.activation(spa[:, :, :], gslT[:, :, :], AF.Abs)
            nc.scalar.activation(spa[:, :, :], spa[:, :, :], AF.Exp, scale=-1.0)
            nc.scalar.activation(spa[:, :, :], spa[:, :, :], AF.Ln, bias=1.0)
            nc.scalar.activation(st3[:, :, :], gslT[:, :, :], AF.Relu)
            nc.vector.tensor_tensor(st3[:, :, :], st3[:, :, :], spa[:, :, :],
                                    AluOpType.add)
            for k in range(1, NK):
                nc.vector.tensor_tensor(g3[:, k, :], g3[:, k - 1, :],
                                        st3[:, k - 1, :], AluOpType.add)

            def recips(d, n):
                dt = cp.tile([128, n, NCH], F32, tag=f"d{d}")
                nc.vector.tensor_tensor(dt[:, :, :], g3[:, d:NK, :],
                                        g3[:, 0:NK - d, :], AluOpType.subtract)
                nc.vector.tensor_scalar_add(dt[:, :, :], dt[:, :, :], EPS)
                r = cp.tile([128, n, NCH], F32, tag=f"r{d}")
                nc.vector.reciprocal(r[:, :, :], dt[:, :, :])
                nr = cp.tile([128, n, NCH], F32, tag=f"nr{d}")
                nc.vector.tensor_scalar_mul(nr[:, :, :], r[:, :, :], -1.0)
                return r, nr

            R1, NR1 = recips(1, NK - 1)   # [128,8,11]
            R2, NR2 = recips(2, NK - 2)   # [128,8,10]
            R3, NR3 = recips(3, NK - 3)   # [128,8,9]

            # biases for the ACT hat ops
            BU = cp.tile([128, 10, NCH], F32)   # -g[j]*R1[j]
            nc.vector.scalar_tensor_tensor(BU[:, :, :], g3[:, 0:10, :], -1.0,
                                           R1[:, 0:10, :],
                                           AluOpType.mult, AluOpType.mult)
            BD = cp.tile([128, 10, NCH], F32)   # g[j+2]*R1[j+1]
            nc.vector.tensor_tensor(BD[:, :, :], g3[:, 2:12, :],
                                    R1[:, 1:11, :], AluOpType.mult)

            # biases for the ACT degree-3 factor ops
            BL3 = cp.tile([128, 8, NCH], F32)   # -g[j]*R3[j]
            nc.vector.scalar_tensor_tensor(BL3[:, :, :], g3[:, 0:8, :], -1.0,
                                           R3[:, 0:8, :],
                                           AluOpType.mult, AluOpType.mult)
            BR3 = cp.tile([128, 8, NCH], F32)   # g[j+4]*R3[j+1]
            nc.vector.tensor_tensor(BR3[:, :, :], g3[:, 4:12, :],
                                    R3[:, 1:9, :], AluOpType.mult)

            rs_t = cp.tile([128, 1], F32)
            nc.gpsimd.dma_start(out=rs_t[:, :], in_=rs[:].to_broadcast((128, 1)))

            psum = [pp.tile([128, BL], F32, tag=f"ps{m}", name=f"ps{m}")
                    for m in range(NCH)]

            xc_tiles = []
            for c in range(NCH):
                xc = xp.tile([128, BL], F32, tag=f"xc{c}")
                nc.sync.dma_start(out=xc[:, :], in_=xt[c * 128:(c + 1) * 128, :])
                xc_tiles.append(xc)

                x16 = sp.tile([128, BL], F16, tag="x16")
                nc.vector.tensor_scalar(x16[:, :], xc[:, :], 1.0,
                                        None, AluOpType.mult)
                UP = bp1.tile([128, 10, BL], F16, tag="up")
                DN = bp1.tile([128, 10, BL], F16, tag="dn")
                for j in range(10):
                    nc.scalar.activation(UP[:, j, :], x16[:, :], AF.Relu,
                                         bias=BU[:, j, c:c+1], scale=R1[:, j, c:c+1])
                    nc.scalar.activation(DN[:, j, :], x16[:, :], AF.Relu,
                                         bias=BD[:, j, c:c+1], scale=NR1[:, j+1, c:c+1])

                L2 = bp2.tile([128, 10, BL], F16, tag="l2")
                R2t = bp2.tile([128, 9, BL], F16, tag="r2")
                L3 = bp3.tile([128, 8, BL], F16, tag="l3")
                R3t = bp3.tile([128, 8, BL], F16, tag="r3")
                for j in range(10):
                    nc.vector.tensor_scalar(L2[:, j, :], x16[:, :],
                                            g3[:, j, c:c+1], R2[:, j, c:c+1],
                                            AluOpType.subtract, AluOpType.mult)
                nc.vector.tensor_scalar(R2t[:, :, :], L2[:, 1:10, :], -1.0,
                                        1.0, AluOpType.mult, AluOpType.add)
                for j in range(3):
                    nc.vector.tensor_scalar(L3[:, j, :], x16[:, :],
                                            g3[:, j, c:c+1], R3[:, j, c:c+1],
                                            AluOpType.subtract, AluOpType.mult)
                for j in range(8):
                    if j >= 3:
                        nc.scalar.activation(L3[:, j, :], x16[:, :], AF.Identity,
                                             bias=BL3[:, j, c:c+1],
                                             scale=R3[:, j, c:c+1])
                    nc.scalar.activation(R3t[:, j, :], x16[:, :], AF.Identity,
                                         bias=BR3[:, j, c:c+1],
                                         scale=NR3[:, j+1, c:c+1])
                sil = sp.tile([128, BL], F16, tag="sil")
                nc.scalar.activation(sil[:, :], x16[:, :], AF.Silu)

                nc.vector.tensor_tensor(UP[:, :, :], UP[:, :, :], DN[:, :, :],
                                        AluOpType.min)   # b1 := UP
                nc.vector.tensor_tensor(L2[:, 0:9, :], L2[:, 0:9, :],
                                        UP[:, 0:9, :], AluOpType.mult)
                nc.vector.tensor_tensor(R2t[:, :, :], R2t[:, :, :],
                                        UP[:, 1:10, :], AluOpType.mult)
                nc.vector.tensor_tensor(L2[:, 0:9, :], L2[:, 0:9, :],
                                        R2t[:, :, :], AluOpType.add)  # b2
                nc.vector.tensor_tensor(L3[:, :, :], L3[:, :, :],
                                        L2[:, 0:8, :], AluOpType.mult)
                nc.vector.tensor_tensor(R3t[:, :, :], R3t[:, :, :],
                                        L2[:, 1:9, :], AluOpType.mult)
                nc.vector.tensor_tensor(L3[:, :, :], L3[:, :, :],
                                        R3t[:, :, :], AluOpType.add)  # b3

                wts = []
                for j in range(9):
                    kc = j * NCH + c
                    wt = wp.tile([128, OUT_DIM], F16, tag="wt", name=f"wt{c}_{j}")
                    nc.sync.dma_start(out=wt[:, :],
                                      in_=w[kc * 128:(kc + 1) * 128, :])
                    wts.append(wt)

                def rhs_of(j):
                    return L3[:, j, :] if j < 8 else sil[:, :]

                if c < NCH - 1:
                    for j in range(9):
                        for m in range(NCH):
                            nc.tensor.matmul(psum[m][:, :],
                                             lhsT=wts[j][:, m * 128:(m + 1) * 128],
                                             rhs=rhs_of(j),
                                             start=(c == 0 and j == 0),
                                             stop=False,
                                             skip_group_check=True)
                else:
                    for m in range(NCH):
                        for j in range(9):
                            nc.tensor.matmul(psum[m][:, :],
                                             lhsT=wts[j][:, m * 128:(m + 1) * 128],
                                             rhs=rhs_of(j),
                                             start=False,
                                             stop=(j == 8),
                                             skip_group_check=True)
                        yt = yp.tile([128, BL], F32, tag="yt", name=f"yt{m}")
                        nc.vector.scalar_tensor_tensor(yt[:, :],
                                                       xc_tiles[m][:, :],
                                                       rs_t[:, :], psum[m][:, :],
                                                       AluOpType.mult,
                                                       AluOpType.add)
                        nc.sync.dma_start(out=y[m * 128:(m + 1) * 128, :],
                                          in_=yt[:, :])

    nc.compile()
    return nc


_NC_CACHE = {}


def kernel(x, coeffs, base_weight, grid_steps_log, grid_start, res_scale,
           _trace=False):
    global LAST_PROFILE

    x = np.asarray(x, dtype=np.float32)
    coeffs = np.asarray(coeffs, dtype=np.float32)
    base_weight = np.asarray(base_weight, dtype=np.float32)
    grid_steps_log = np.asarray(grid_steps_log, dtype=np.float32)
    grid_start = np.asarray(grid_start, dtype=np.float32)
    res_scale = np.asarray(res_scale, dtype=np.float32)

    # ---- host-side grid analysis (float64) ----
    steps64 = np.logaddexp(0.0, grid_steps_log.astype(np.float64))  # softplus
    g0_64 = grid_start.astype(np.float64)[:, 0]
    h = float(steps64.mean())
    g0 = float(g0_64.mean())
    uniform = (np.abs(steps64 - h).max() <= 1e-6 * max(abs(h), 1e-12)
               and np.abs(g0_64 - g0).max() <= 1e-6 and h > 0)

    xT = np.ascontiguousarray(x.T)                                # [in, B]
    rs_r = res_scale.reshape(1, 1)

    # weight blocks; block j=8 is base_weight.T
    wj = coeffs.reshape(OUT_DIM, IN_DIM, 8).transpose(2, 1, 0)    # [8, in, out]
    if uniform:
        wj = wj * (1.0 / 6.0)        # fold the 1/6 of the cardinal spline
    big_w = np.concatenate([wj, base_weight.T[None]], axis=0)     # [9, in, out]
    if uniform:
        # chunk-major row order: row (c*9 + j)*128 + p  (one DMA per chunk)
        big_w = (big_w.reshape(9, NCH, 128, OUT_DIM).transpose(1, 0, 2, 3)
                 .reshape(9 * IN_DIM, OUT_DIM))
    else:
        # j-major row order: k = j*IN_DIM + i
        big_w = big_w.reshape(9 * IN_DIM, OUT_DIM)
    big_w = np.ascontiguousarray(big_w, dtype=np.float16)

    if uniform:
        key = ("uniform", round(1.0 / h, 9), round(-g0 / h, 9))
        if key not in _NC_CACHE:
            _NC_CACHE.clear()
            _NC_CACHE[key] = _build_nc_uniform(1.0 / h, -g0 / h)
        nc = _NC_CACHE[key]
        in_maps = [{
            "xt": np.ascontiguousarray(xT[:, c * BL:(c + 1) * BL]),
            "xb": np.ascontiguousarray(x[c * BL:(c + 1) * BL, :]),
            "w": big_w,
            "rs": rs_r,
        } for c in range(N_CORES)]
    else:
        key = ("general",)
        if key not in _NC_CACHE:
            _NC_CACHE.clear()
            _NC_CACHE[key] = _build_nc_general()
        nc = _NC_CACHE[key]
        gsl_r = np.ascontiguousarray(
            grid_steps_log.reshape(NCH, 128, NK - 1).transpose(1, 2, 0)
            .reshape(128, (NK - 1) * NCH))
        gst_r = np.ascontiguousarray(grid_start.reshape(NCH, 128).T)
        in_maps = [{
            "xt": np.ascontiguousarray(xT[:, c * BL:(c + 1) * BL]),
            "w": big_w,
            "gsl": gsl_r,
            "gst": gst_r,
            "rs": rs_r,
        } for c in range(N_CORES)]

    res = run_bass_kernel_spmd(nc, in_maps, core_ids=list(range(N_CORES)),
                               trace=_trace)
    LAST_PROFILE = {
        "exec_time_ns": res.exec_time_ns,
        "mean_exec_time_ns": res.mean_exec_time_ns,
        "max_exec_time_core_id": res.max_exec_time_core_id,
        "profile_json": res.profile_json,
        "instructions_and_trace": res.instructions_and_trace,
    }

    if uniform:
        out = np.concatenate([r["y"] for r in res.results], axis=0)  # [B, out]
    else:
        out = np.concatenate([r["y"].T for r in res.results], axis=0)
    return np.ascontiguousarray(out.astype(np.float32))


# revision 23
# speedup vs baseline: 1.2737x; 1.2281x over previous
"""BSpline KAN layer (grid_size=5, spline_order=3) on 8 Trainium2 NeuronCores.

Strategy (data-parallel over batch):
  - Each core gets B_local = 512 rows of x, replicated weights.
  - Layout on-chip: in-dim on partitions (8 chunks of 128), batch on free dim.

Fast path (host-detected uniform identical grid, which setup_inputs produces):
  With u = (x - g0)/h the cubic bases are the cardinal B-spline b3_j = S(u-j).
  Using the two-sided truncated-power form with m_j = 2 - |u - (j+2)|:
      b3_j = (1/6)*relu(m_j)^3 - (2/3)*relu(m_j - 1)^3
  (exact: S is symmetric about its center; for m <= 2 the remaining truncated
  terms vanish, and all values are bounded by 8 so fp16 is safe).
  Per in-chunk: 8 narrow ACT Abs ops (immediate scale/bias) + 1 ACT Square +
  7 wide DVE ops produce the full [128, 8, 512] basis stack; the 1/6 is folded
  into the spline weights on the host.  ACT ~10us/chunk, DVE ~11us/chunk, both
  below the tensor engine's ~18us/chunk matmul stream.

  - Spline contraction as matmul with k-order j-major: k = j*1024 + i, so the
    j-stacked basis tiles are directly the matmul rhs. silu(x) @ base_weight.T
    is folded in as a 9th "basis" with base_weight as its weight block.
  - All 8 PSUM banks accumulate the 8 out-chunks across the whole contraction;
    epilogue adds res_scale * x and stores y[out, batch] (host transposes).

Fallback path (general grids): Cox-de Boor recursion kernel (unchanged from
the general implementation; correct for any grid).
Precision: fp16 bases/weights, fp32 accumulation.
"""

import numpy as np

import concourse.bass as bass
from concourse import bacc
import concourse.mybir as mybir
import concourse.tile as tile
from concourse.alu_op_type import AluOpType
from concourse.bass_utils import run_bass_kernel_spmd

F32 = mybir.dt.float32
F16 = mybir.dt.float16
AF = mybir.ActivationFunctionType

IN_DIM = 1024
OUT_DIM = 1024
BATCH = 4096
N_CORES = 8
BL = BATCH // N_CORES        # 512 batch rows per core
NCH = IN_DIM // 128          # 8 in-dim chunks
NK = 12                      # knots per dim
EPS = 1e-8

LAST_PROFILE = {}


def _build_nc_uniform(inv_h, nu0):
    """Uniform-grid kernel.  u = x*inv_h + nu0 (nu0 = -g0/h);
    A_j = |u - (j+2)|, m_j = 2 - A_j,
    B3_j = 6*b3_j = relu(m_j)^3 - 4*relu(m_j-1)^3  (weights pre-divided by 6).
    Computed as: PT = min(A-2,0) = -relu(m);  QT = min(A-1,0) = -relu(m-1);
    SP = PT^2*PT = -relu(m)^3 ; SQ = -relu(m-1)^3 ; B3 = 4*SQ - SP.

    Matmuls run bases-stationary / weights-moving: lhsT = B3[in128, batch128]
    so each stationary serves both 512-wide out-halves (halves LDWEIGHTS).
    PSUM bank (bt, oh) = [128 batch, 512 out]; y is emitted [batch, out]."""
    nc = bacc.Bacc("TRN2", target_bir_lowering=False)

    xt = nc.dram_tensor("xt", [IN_DIM, BL], F32, kind="ExternalInput")
    xb = nc.dram_tensor("xb", [BL, IN_DIM], F32, kind="ExternalInput")
    w = nc.dram_tensor("w", [9 * IN_DIM, OUT_DIM], F16, kind="ExternalInput")
    rs = nc.dram_tensor("rs", [1, 1], F32, kind="ExternalInput")
    y = nc.dram_tensor("y", [BL, OUT_DIM], F32, kind="ExternalOutput")

    NBT = BL // 128              # 4 batch tiles
    NOH = OUT_DIM // 512         # 2 out halves

    with tile.TileContext(nc) as tc:
        with (
            tc.tile_pool(name="const", bufs=1) as cp,
            tc.tile_pool(name="xres", bufs=1) as xp,
            tc.tile_pool(name="sil", bufs=2) as sp,
            tc.tile_pool(name="abs", bufs=2) as apl,
            tc.tile_pool(name="cube", bufs=2) as bp,
            tc.tile_pool(name="b3", bufs=2) as b3p,
            tc.tile_pool(name="wts", bufs=3) as wp,
            tc.tile_pool(name="yout", bufs=4) as yp,
            tc.tile_pool(name="psum", bufs=1, space="PSUM") as pp,
        ):
            rs_t = cp.tile([128, 1], F32)
            nc.gpsimd.dma_start(out=rs_t[:, :], in_=rs[:].to_broadcast((128, 1)))

            # per-j ACT biases nu0 - (j+2) as [128,1] columns (no const-AP
            # registration for arbitrary floats); memset on the idle DVE so
            # they are ready before the first Abs
            bias_t = cp.tile([128, 8], F32)
            for j in range(8):
                nc.vector.memset(bias_t[:, j:j + 1], float(nu0 - (j + 2)))

            # dummy Silu: triggers the one-time load of silu_and_others (the
            # only table set needed: it contains abs/square/silu/copy) while
            # the first x tile is still streaming in
            warm = cp.tile([128, 2], F32)
            nc.vector.memset(warm[:, 0:1], 0.0)
            nc.scalar.activation(warm[:, 1:2], warm[:, 0:1], AF.Silu)

            # PSUM accumulators: bank (bt, oh) = [128 batch, 512 out]
            psum = [pp.tile([128, 512], F32, tag=f"ps{b}", name=f"ps{b}")
                    for b in range(NBT * NOH)]

            HB = 4 * BL          # half-stack width (j 0-3 | 4-7)
            xb_tiles = []

            for c in range(NCH):
                xc = xp.tile([128, BL], F32, tag=f"xc{c}")
                nc.sync.dma_start(out=xc[:, :],
                                  in_=xt[c * 128:(c + 1) * 128, :])

                # flat [128, 8*BL] stacks: 1-D free dim so wide DVE ops pay the
                # SBUF inter-instruction bubble once, not per 512-row
                A = apl.tile([128, 8 * BL], F16, tag="A")
                sil = sp.tile([128, BL], F16, tag="sil")
                if c == 0:
                    # silu first: the j=8 matmul block starts the tensor
                    # engine while the Abs/cube chain is still filling
                    nc.scalar.activation(sil[:, :], xc[:, :], AF.Silu)
                for j in range(8):
                    nc.scalar.activation(A[:, j * BL:(j + 1) * BL], xc[:, :],
                                         AF.Abs, bias=bias_t[:, j:j + 1],
                                         scale=float(inv_h))
                if c > 0:
                    nc.scalar.activation(sil[:, :], xc[:, :], AF.Silu)

                PT = bp.tile([128, 8 * BL], F16, tag="PT")
                QT = bp.tile([128, 8 * BL], F16, tag="QT")
                SP = bp.tile([128, 8 * BL], F16, tag="SP")
                SQ = bp.tile([128, 8 * BL], F16, tag="SQ")
                B3 = b3p.tile([128, 8 * BL], F16, tag="B3")

                # per slice s: PT = -relu(m), QT = -relu(m-1) (4x tensor_scalar)
                # SQ = -relu(m-1)^3 via 2 DVE mults; SP = relu(m)^2 (ACT Square
                # in steady state, DVE for the latency-critical first chunk),
                # cubed on DVE; B3 = 4*SQ - SP = 6*b3.
                def basis_ops(s, dve_square):
                    nc.vector.tensor_scalar(PT[:, s], A[:, s], 2.0, 0.0,
                                            AluOpType.subtract, AluOpType.min)
                    nc.vector.tensor_scalar(QT[:, s], A[:, s], 1.0, 0.0,
                                            AluOpType.subtract, AluOpType.min)
                    nc.vector.tensor_tensor(SQ[:, s], QT[:, s], QT[:, s],
                                            AluOpType.mult)
                    nc.vector.tensor_tensor(SQ[:, s], SQ[:, s], QT[:, s],
                                            AluOpType.mult)
                    if dve_square:
                        nc.vector.tensor_tensor(SP[:, s], PT[:, s], PT[:, s],
                                                AluOpType.mult)
                    else:
                        nc.scalar.activation(SP[:, s], PT[:, s], AF.Square)
                    nc.vector.tensor_tensor(SP[:, s], SP[:, s], PT[:, s],
                                            AluOpType.mult)
                    nc.vector.scalar_tensor_tensor(B3[:, s], SQ[:, s], 4.0,
                                                   SP[:, s],
                                                   AluOpType.mult,
                                                   AluOpType.subtract)

                if c == 0:
                    # quarter-split; all-DVE only for the latency-critical
                    # first quarter, ACT squares after (early DVE debt feeds
                    # the chunk-1/2 tensor stalls otherwise)
                    for q in range(4):
                        basis_ops(slice(q * 2 * BL, (q + 1) * 2 * BL), q == 0)
                elif c == 1:
                    for q in range(4):
                        basis_ops(slice(q * 2 * BL, (q + 1) * 2 * BL), False)
                else:
                    for h in range(2):
                        basis_ops(slice(h * HB, (h + 1) * HB), False)

                # weight blocks (8 spline j's + silu/base_weight), moving
                # operand.  Host layout is chunk-major: rows (c*9+j)*128+p.
                # Per-block DMAs in consumption order, alternating between the
                # sync and scalar HW queues: doubles weight bandwidth while
                # matmuls unblock block-by-block.
                JORDER = ([8, 0, 1, 2, 3, 4, 5, 6, 7] if c == 0
                          else [0, 1, 2, 3, 8, 4, 5, 6, 7])
                wt = wp.tile([128, 9 * OUT_DIM], F16, tag="wt", name=f"wt{c}")
                base = c * 9 * 128
                for j in JORDER:
                    nc.sync.dma_start(
                        out=wt[:, j * OUT_DIM:(j + 1) * OUT_DIM],
                        in_=w[base + j * 128:base + (j + 1) * 128, :])

                if c == NCH - 1:
                    # batch-major x tiles for the residual epilogue (only
                    # needed now; keeps early DMA bandwidth for weights)
                    for bt in range(NBT):
                        xbt = cp.tile([128, IN_DIM], F32, tag=f"xb{bt}")
                        nc.sync.dma_start(out=xbt[:, :],
                                            in_=xb[bt * 128:(bt + 1) * 128, :])
                        xb_tiles.append(xbt)

                def stat_of(j, bt):
                    if j < 8:
                        return B3[:, j * BL + bt * 128:j * BL + (bt + 1) * 128]
                    return sil[:, bt * 128:(bt + 1) * 128]

                def rhs_of(j, oh):
                    return wt[:, j * OUT_DIM + oh * 512:
                              j * OUT_DIM + (oh + 1) * 512]

                # consume in production order
                if c < NCH - 1:
                    for j in JORDER:
                        for bt in range(NBT):
                            for oh in range(NOH):
                                nc.tensor.matmul(
                                    psum[bt * NOH + oh][:, :],
                                    lhsT=stat_of(j, bt),
                                    rhs=rhs_of(j, oh),
                                    start=(c == 0 and j == 8),
                                    stop=False,
                                    skip_group_check=True)
                else:
                    # last chunk: bt-outer so each PSUM bank pair finishes
                    # early and its epilogue overlaps the remaining matmuls
                    for bt in range(NBT):
                        for j in JORDER:
                            for oh in range(NOH):
                                nc.tensor.matmul(
                                    psum[bt * NOH + oh][:, :],
                                    lhsT=stat_of(j, bt),
                                    rhs=rhs_of(j, oh),
                                    start=False,
                                    stop=(j == 7),
                                    skip_group_check=True)
                        for oh in range(NOH):
                            # ScalarE drains PSUM (it sits closest to PSUM),
                            # DVE adds the residual all-SBUF
                            yt = yp.tile([128, 512], F32, tag="yt",
                                         name=f"yt{bt}_{oh}")
                            nc.scalar.activation(yt[:, :],
                                                 psum[bt * NOH + oh][:, :],
                                                 AF.Copy)
                            nc.vector.scalar_tensor_tensor(
                                yt[:, :],
                                xb_tiles[bt][:, oh * 512:(oh + 1) * 512],
                                rs_t[:, :], yt[:, :],
                                AluOpType.mult, AluOpType.add)
                            nc.sync.dma_start(
                                out=y[bt * 128:(bt + 1) * 128,
                                      oh * 512:(oh + 1) * 512],
                                in_=yt[:, :])

    nc.compile()
    return nc


def _build_nc_general():
    """General-grid fallback: Cox-de Boor recursion on device."""
    nc = bacc.Bacc("TRN2", target_bir_lowering=False)

    xt = nc.dram_tensor("xt", [IN_DIM, BL], F32, kind="ExternalInput")
    w = nc.dram_tensor("w", [9 * IN_DIM, OUT_DIM], F16, kind="ExternalInput")
    gsl = nc.dram_tensor("gsl", [128, NCH * (NK - 1)], F32, kind="ExternalInput")
    gst = nc.dram_tensor("gst", [128, NCH], F32, kind="ExternalInput")
    rs = nc.dram_tensor("rs", [1, 1], F32, kind="ExternalInput")
    y = nc.dram_tensor("y", [OUT_DIM, BL], F32, kind="ExternalOutput")

    with tile.TileContext(nc) as tc:
        with (
            tc.tile_pool(name="const", bufs=1) as cp,
            tc.tile_pool(name="xres", bufs=1) as xp,
            tc.tile_pool(name="small", bufs=4) as sp,
            tc.tile_pool(name="updn", bufs=2) as bp1,
            tc.tile_pool(name="lr2", bufs=2) as bp2,
            tc.tile_pool(name="lr3", bufs=3) as bp3,
            tc.tile_pool(name="wts", bufs=12) as wp,
            tc.tile_pool(name="yout", bufs=4) as yp,
            tc.tile_pool(name="psum", bufs=1, space="PSUM") as pp,
        ):
            # ---------------- grid preparation (once) ----------------
            gslT = cp.tile([128, NK - 1, NCH], F32)
            nc.gpsimd.dma_start(out=gslT[:, :, :],
                                in_=gsl[:, :].rearrange("p (k c) -> p k c", c=NCH))
            g3 = cp.tile([128, NK, NCH], F32)
            nc.gpsimd.dma_start(out=g3[:, 0, :], in_=gst[:, :])

            # softplus(v) = relu(v) + ln(1 + exp(-|v|))
            st3 = cp.tile([128, NK - 1, NCH], F32)
            spa = cp.tile([128, NK - 1, NCH], F32)
            nc.scalar.activation(spa[:, :, :], gslT[:, :, :], AF.Abs)
            nc.scalar.activation(spa[:, :, :], spa[:, :, :], AF.Exp, scale=-1.0)
            nc.scalar.activation(spa[:, :, :], spa[:, :, :], AF.Ln, bias=1.0)
            nc.scalar.activation(st3[:, :, :], gslT[:, :, :], AF.Relu)
            nc.vector.tensor_tensor(st3[:, :, :], st3[:, :, :], spa[:, :, :],
                                    AluOpType.add)
            for k in range(1, NK):
                nc.vector.tensor_tensor(g3[:, k, :], g3[:, k - 1, :],
                                        st3[:, k - 1, :], AluOpType.add)

            def recips(d, n):
                dt = cp.tile([128, n, NCH], F32, tag=f"d{d}")
                nc.vector.tensor_tensor(dt[:, :, :], g3[:, d:NK, :],
                                        g3[:, 0:NK - d, :], AluOpType.subtract)
                nc.vector.tensor_scalar_add(dt[:, :, :], dt[:, :, :], EPS)
                r = cp.tile([128, n, NCH], F32, tag=f"r{d}")
                nc.vector.reciprocal(r[:, :, :], dt[:, :, :])
                nr = cp.tile([128, n, NCH], F32, tag=f"nr{d}")
                nc.vector.tensor_scalar_mul(nr[:, :, :], r[:, :, :], -1.0)
                return r, nr

            R1, NR1 = recips(1, NK - 1)   # [128,8,11]
            R2, NR2 = recips(2, NK - 2)   # [128,8,10]
            R3, NR3 = recips(3, NK - 3)   # [128,8,9]

            # biases for the ACT hat ops
            BU = cp.tile([128, 10, NCH], F32)   # -g[j]*R1[j]
            nc.vector.scalar_tensor_tensor(BU[:, :, :], g3[:, 0:10, :], -1.0,
                                           R1[:, 0:10, :],
                                           AluOpType.mult, AluOpType.mult)
            BD = cp.tile([128, 10, NCH], F32)   # g[j+2]*R1[j+1]
            nc.vector.tensor_tensor(BD[:, :, :], g3[:, 2:12, :],
                                    R1[:, 1:11, :], AluOpType.mult)

            # biases for the ACT degree-3 factor ops
            BL3 = cp.tile([128, 8, NCH], F32)   # -g[j]*R3[j]
            nc.vector.scalar_tensor_tensor(BL3[:, :, :], g3[:, 0:8, :], -1.0,
                                           R3[:, 0:8, :],
                                           AluOpType.mult, AluOpType.mult)
            BR3 = cp.tile([128, 8, NCH], F32)   # g[j+4]*R3[j+1]
            nc.vector.tensor_tensor(BR3[:, :, :], g3[:, 4:12, :],
                                    R3[:, 1:9, :], AluOpType.mult)

            rs_t = cp.tile([128, 1], F32)
            nc.gpsimd.dma_start(out=rs_t[:, :], in_=rs[:].to_broadcast((128, 1)))

            psum = [pp.tile([128, BL], F32, tag=f"ps{m}", name=f"ps{m}")
                    for m in range(NCH)]

            xc_tiles = []
            for c in range(NCH):
                xc = xp.tile([128, BL], F32, tag=f"xc{c}")
                nc.sync.dma_start(out=xc[:, :], in_=xt[c * 128:(c + 1) * 128, :])
                xc_tiles.append(xc)

                x16 = sp.tile([128, BL], F16, tag="x16")
                nc.vector.tensor_scalar(x16[:, :], xc[:, :], 1.0,
                                        None, AluOpType.mult)
                UP = bp1.tile([128, 10, BL], F16, tag="up")
                DN = bp1.tile([128, 10, BL], F16, tag="dn")
                for j in range(10):
                    nc.scalar.activation(UP[:, j, :], x16[:, :], AF.Relu,
                                         bias=BU[:, j, c:c+1], scale=R1[:, j, c:c+1])
                    nc.scalar.activation(DN[:, j, :], x16[:, :], AF.Relu,
                                         bias=BD[:, j, c:c+1], scale=NR1[:, j+1, c:c+1])

                L2 = bp2.tile([128, 10, BL], F16, tag="l2")
                R2t = bp2.tile([128, 9, BL], F16, tag="r2")
                L3 = bp3.tile([128, 8, BL], F16, tag="l3")
                R3t = bp3.tile([128, 8, BL], F16, tag="r3")
                for j in range(10):
                    nc.vector.tensor_scalar(L2[:, j, :], x16[:, :],
                                            g3[:, j, c:c+1], R2[:, j, c:c+1],
                                            AluOpType.subtract, AluOpType.mult)
                nc.vector.tensor_scalar(R2t[:, :, :], L2[:, 1:10, :], -1.0,
                                        1.0, AluOpType.mult, AluOpType.add)
                for j in range(3):
                    nc.vector.tensor_scalar(L3[:, j, :], x16[:, :],
                                            g3[:, j, c:c+1], R3[:, j, c:c+1],
                                            AluOpType.subtract, AluOpType.mult)
                for j in range(8):
                    if j >= 3:
                        nc.scalar.activation(L3[:, j, :], x16[:, :], AF.Identity,
                                             bias=BL3[:, j, c:c+1],
                                             scale=R3[:, j, c:c+1])
                    nc.scalar.activation(R3t[:, j, :], x16[:, :], AF.Identity,
                                         bias=BR3[:, j, c:c+1],
                                         scale=NR3[:, j+1, c:c+1])
                sil = sp.tile([128, BL], F16, tag="sil")
                nc.scalar.activation(sil[:, :], x16[:, :], AF.Silu)

                nc.vector.tensor_tensor(UP[:, :, :], UP[:, :, :], DN[:, :, :],
                                        AluOpType.min)   # b1 := UP
                nc.vector.tensor_tensor(L2[:, 0:9, :], L2[:, 0:9, :],
                                        UP[:, 0:9, :], AluOpType.mult)
                nc.vector.tensor_tensor(R2t[:, :, :], R2t[:, :, :],
                                        UP[:, 1:10, :], AluOpType.mult)
                nc.vector.tensor_tensor(L2[:, 0:9, :], L2[:, 0:9, :],
                                        R2t[:, :, :], AluOpType.add)  # b2
                nc.vector.tensor_tensor(L3[:, :, :], L3[:, :, :],
                                        L2[:, 0:8, :], AluOpType.mult)
                nc.vector.tensor_tensor(R3t[:, :, :], R3t[:, :, :],
                                        L2[:, 1:9, :], AluOpType.mult)
                nc.vector.tensor_tensor(L3[:, :, :], L3[:, :, :],
                                        R3t[:, :, :], AluOpType.add)  # b3

                wts = []
                for j in range(9):
                    kc = j * NCH + c
                    wt = wp.tile([128, OUT_DIM], F16, tag="wt", name=f"wt{c}_{j}")
                    nc.sync.dma_start(out=wt[:, :],
                                      in_=w[kc * 128:(kc + 1) * 128, :])
                    wts.append(wt)

                def rhs_of(j):
                    return L3[:, j, :] if j < 8 else sil[:, :]

                if c < NCH - 1:
                    for j in range(9):
                        for m in range(NCH):
                            nc.tensor.matmul(psum[m][:, :],
                                             lhsT=wts[j][:, m * 128:(m + 1) * 128],
                                             rhs=rhs_of(j),
                                             start=(c == 0 and j == 0),
                                             stop=False,
                                             skip_group_check=True)
                else:
                    for m in range(NCH):
                        for j in range(9):
                            nc.tensor.matmul(psum[m][:, :],
                                             lhsT=wts[j][:, m * 128:(m + 1) * 128],
                                             rhs=rhs_of(j),
                                             start=False,
                                             stop=(j == 8),
                                             skip_group_check=True)
                        yt = yp.tile([128, BL], F32, tag="yt", name=f"yt{m}")
                        nc.vector.scalar_tensor_tensor(yt[:, :],
                                                       xc_tiles[m][:, :],
                                                       rs_t[:, :], psum[m][:, :],
                                                       AluOpType.mult,
                                                       AluOpType.add)
                        nc.sync.dma_start(out=y[m * 128:(m + 1) * 128, :],
                                          in_=yt[:, :])

    nc.compile()
    return nc


_NC_CACHE = {}


def kernel(x, coeffs, base_weight, grid_steps_log, grid_start, res_scale,
           _trace=False):
    global LAST_PROFILE

    x = np.asarray(x, dtype=np.float32)
    coeffs = np.asarray(coeffs, dtype=np.float32)
    base_weight = np.asarray(base_weight, dtype=np.float32)
    grid_steps_log = np.asarray(grid_steps_log, dtype=np.float32)
    grid_start = np.asarray(grid_start, dtype=np.float32)
    res_scale = np.asarray(res_scale, dtype=np.float32)

    # ---- host-side grid analysis (float64) ----
    steps64 = np.logaddexp(0.0, grid_steps_log.astype(np.float64))  # softplus
    g0_64 = grid_start.astype(np.float64)[:, 0]
    h = float(steps64.mean())
    g0 = float(g0_64.mean())
    uniform = (np.abs(steps64 - h).max() <= 1e-6 * max(abs(h), 1e-12)
               and np.abs(g0_64 - g0).max() <= 1e-6 and h > 0)

    xT = np.ascontiguousarray(x.T)                                # [in, B]
    rs_r = res_scale.reshape(1, 1)

    # weight blocks; block j=8 is base_weight.T
    wj = coeffs.reshape(OUT_DIM, IN_DIM, 8).transpose(2, 1, 0)    # [8, in, out]
    if uniform:
        wj = wj * (1.0 / 6.0)        # fold the 1/6 of the cardinal spline
    big_w = np.concatenate([wj, base_weight.T[None]], axis=0)     # [9, in, out]
    if uniform:
        # chunk-major row order: row (c*9 + j)*128 + p  (one DMA per chunk)
        big_w = (big_w.reshape(9, NCH, 128, OUT_DIM).transpose(1, 0, 2, 3)
                 .reshape(9 * IN_DIM, OUT_DIM))
    else:
        # j-major row order: k = j*IN_DIM + i
        big_w = big_w.reshape(9 * IN_DIM, OUT_DIM)
    big_w = np.ascontiguousarray(big_w, dtype=np.float16)

    if uniform:
        key = ("uniform", round(1.0 / h, 9), round(-g0 / h, 9))
        if key not in _NC_CACHE:
            _NC_CACHE.clear()
            _NC_CACHE[key] = _build_nc_uniform(1.0 / h, -g0 / h)
        nc = _NC_CACHE[key]
        in_maps = [{
            "xt": np.ascontiguousarray(xT[:, c * BL:(c + 1) * BL]),
            "xb": np.ascontiguousarray(x[c * BL:(c + 1) * BL, :]),
            "w": big_w,
            "rs": rs_r,
        } for c in range(N_CORES)]
    else:
        key = ("general",)
        if key not in _NC_CACHE:
            _NC_CACHE.clear()
            _NC_CACHE[key] = _build_nc_general()
        nc = _NC_CACHE[key]
        gsl_r = np.ascontiguousarray(
            grid_steps_log.reshape(NCH, 128, NK - 1).transpose(1, 2, 0)
            .reshape(128, (NK - 1) * NCH))
        gst_r = np.ascontiguousarray(grid_start.reshape(NCH, 128).T)
        in_maps = [{
            "xt": np.ascontiguousarray(xT[:, c * BL:(c + 1) * BL]),
            "w": big_w,
            "gsl": gsl_r,
            "gst": gst_r,
            "rs": rs_r,
        } for c in range(N_CORES)]

    res = run_bass_kernel_spmd(nc, in_maps, core_ids=list(range(N_CORES)),
                               trace=_trace)
    LAST_PROFILE = {
        "exec_time_ns": res.exec_time_ns,
        "mean_exec_time_ns": res.mean_exec_time_ns,
        "max_exec_time_core_id": res.max_exec_time_core_id,
        "profile_json": res.profile_json,
        "instructions_and_trace": res.instructions_and_trace,
    }

    if uniform:
        out = np.concatenate([r["y"] for r in res.results], axis=0)  # [B, out]
    else:
        out = np.concatenate([r["y"].T for r in res.results], axis=0)
    return np.ascontiguousarray(out.astype(np.float32))


# revision 24
# speedup vs baseline: 1.2904x; 1.0131x over previous
"""BSpline KAN layer (grid_size=5, spline_order=3) on 8 Trainium2 NeuronCores.

Strategy (data-parallel over batch):
  - Each core gets B_local = 512 rows of x, replicated weights.
  - Layout on-chip: in-dim on partitions (8 chunks of 128), batch on free dim.

Fast path (host-detected uniform identical grid, which setup_inputs produces):
  With u = (x - g0)/h the cubic bases are the cardinal B-spline b3_j = S(u-j).
  Using the two-sided truncated-power form with m_j = 2 - |u - (j+2)|:
      b3_j = (1/6)*relu(m_j)^3 - (2/3)*relu(m_j - 1)^3
  (exact: S is symmetric about its center; for m <= 2 the remaining truncated
  terms vanish, and all values are bounded by 8 so fp16 is safe).
  Per in-chunk: 8 narrow ACT Abs ops (immediate scale/bias) + 1 ACT Square +
  7 wide DVE ops produce the full [128, 8, 512] basis stack; the 1/6 is folded
  into the spline weights on the host.  ACT ~10us/chunk, DVE ~11us/chunk, both
  below the tensor engine's ~18us/chunk matmul stream.

  - Spline contraction as matmul with k-order j-major: k = j*1024 + i, so the
    j-stacked basis tiles are directly the matmul rhs. silu(x) @ base_weight.T
    is folded in as a 9th "basis" with base_weight as its weight block.
  - All 8 PSUM banks accumulate the 8 out-chunks across the whole contraction;
    epilogue adds res_scale * x and stores y[out, batch] (host transposes).

Fallback path (general grids): Cox-de Boor recursion kernel (unchanged from
the general implementation; correct for any grid).
Precision: fp16 bases/weights, fp32 accumulation.
"""

import numpy as np

import concourse.bass as bass
from concourse import bacc
import concourse.mybir as mybir
import concourse.tile as tile
from concourse.alu_op_type import AluOpType
from concourse.bass_utils import run_bass_kernel_spmd

F32 = mybir.dt.float32
F16 = mybir.dt.float16
AF = mybir.ActivationFunctionType

IN_DIM = 1024
OUT_DIM = 1024
BATCH = 4096
N_CORES = 8
BL = BATCH // N_CORES        # 512 batch rows per core
NCH = IN_DIM // 128          # 8 in-dim chunks
NK = 12                      # knots per dim
EPS = 1e-8

LAST_PROFILE = {}


def _build_nc_uniform(inv_h, nu0):
    """Uniform-grid kernel.  u = x*inv_h + nu0 (nu0 = -g0/h);
    A_j = |u - (j+2)|, m_j = 2 - A_j,
    B3_j = 6*b3_j = relu(m_j)^3 - 4*relu(m_j-1)^3  (weights pre-divided by 6).
    Computed as: PT = min(A-2,0) = -relu(m);  QT = min(A-1,0) = -relu(m-1);
    SP = PT^2*PT = -relu(m)^3 ; SQ = -relu(m-1)^3 ; B3 = 4*SQ - SP.

    Matmuls run bases-stationary / weights-moving: lhsT = B3[in128, batch128]
    so each stationary serves both 512-wide out-halves (halves LDWEIGHTS).
    PSUM bank (bt, oh) = [128 batch, 512 out]; y is emitted [batch, out]."""
    nc = bacc.Bacc("TRN2", target_bir_lowering=False)

    xt = nc.dram_tensor("xt", [IN_DIM, BL], F32, kind="ExternalInput")
    xb = nc.dram_tensor("xb", [BL, IN_DIM], F32, kind="ExternalInput")
    w = nc.dram_tensor("w", [9 * IN_DIM, OUT_DIM], F16, kind="ExternalInput")
    rs = nc.dram_tensor("rs", [1, 1], F32, kind="ExternalInput")
    y = nc.dram_tensor("y", [BL, OUT_DIM], F32, kind="ExternalOutput")

    NBT = BL // 128              # 4 batch tiles
    NOH = OUT_DIM // 512         # 2 out halves

    with tile.TileContext(nc) as tc:
        with (
            tc.tile_pool(name="const", bufs=1) as cp,
            tc.tile_pool(name="xres", bufs=1) as xp,
            tc.tile_pool(name="sil", bufs=2) as sp,
            tc.tile_pool(name="abs", bufs=2) as apl,
            tc.tile_pool(name="cube", bufs=2) as bp,
            tc.tile_pool(name="b3", bufs=2) as b3p,
            tc.tile_pool(name="wts", bufs=3) as wp,
            tc.tile_pool(name="yout", bufs=4) as yp,
            tc.tile_pool(name="psum", bufs=1, space="PSUM") as pp,
        ):
            rs_t = cp.tile([128, 1], F32)
            nc.gpsimd.dma_start(out=rs_t[:, :], in_=rs[:].to_broadcast((128, 1)))

            # per-j ACT biases nu0 - (j+2) as [128,1] columns (no const-AP
            # registration for arbitrary floats); memset on the idle DVE so
            # they are ready before the first Abs
            bias_t = cp.tile([128, 8], F32)
            for j in range(8):
                nc.vector.memset(bias_t[:, j:j + 1], float(nu0 - (j + 2)))

            # dummy Silu: triggers the one-time load of silu_and_others (the
            # only table set needed: it contains abs/square/silu/copy) while
            # the first x tile is still streaming in
            warm = cp.tile([128, 2], F32)
            nc.vector.memset(warm[:, 0:1], 0.0)
            nc.scalar.activation(warm[:, 1:2], warm[:, 0:1], AF.Silu)

            # PSUM accumulators: bank (bt, oh) = [128 batch, 512 out]
            psum = [pp.tile([128, 512], F32, tag=f"ps{b}", name=f"ps{b}")
                    for b in range(NBT * NOH)]

            # PE clock pre-warm: dummy matmuls on zeroed tiles while the
            # first x/weight tiles stream in (results discarded by the real
            # group's start=True reset)
            dummy = cp.tile([128, BL], F16, tag="dummy")
            nc.vector.memset(dummy[:, :], 0.0)
            for k in range(14):
                nc.tensor.matmul(psum[0][:, :], lhsT=dummy[:, 0:128],
                                 rhs=dummy[:, :], start=(k == 0),
                                 stop=(k == 13), skip_group_check=True)

            HB = 4 * BL          # half-stack width (j 0-3 | 4-7)
            xb_tiles = []

            for c in range(NCH):
                xc = xp.tile([128, BL], F32, tag=f"xc{c}")
                nc.sync.dma_start(out=xc[:, :],
                                  in_=xt[c * 128:(c + 1) * 128, :])

                # flat [128, 8*BL] stacks: 1-D free dim so wide DVE ops pay the
                # SBUF inter-instruction bubble once, not per 512-row
                A = apl.tile([128, 8 * BL], F16, tag="A")
                sil = sp.tile([128, BL], F16, tag="sil")
                if c == 0:
                    # silu first: the j=8 matmul block starts the tensor
                    # engine while the Abs/cube chain is still filling
                    nc.scalar.activation(sil[:, :], xc[:, :], AF.Silu)
                for j in range(8):
                    nc.scalar.activation(A[:, j * BL:(j + 1) * BL], xc[:, :],
                                         AF.Abs, bias=bias_t[:, j:j + 1],
                                         scale=float(inv_h))
                if c > 0:
                    nc.scalar.activation(sil[:, :], xc[:, :], AF.Silu)

                PT = bp.tile([128, 8 * BL], F16, tag="PT")
                QT = bp.tile([128, 8 * BL], F16, tag="QT")
                SP = bp.tile([128, 8 * BL], F16, tag="SP")
                SQ = bp.tile([128, 8 * BL], F16, tag="SQ")
                B3 = b3p.tile([128, 8 * BL], F16, tag="B3")

                # per slice s: PT = -relu(m), QT = -relu(m-1) (4x tensor_scalar)
                # SQ = -relu(m-1)^3 via 2 DVE mults; SP = relu(m)^2 (ACT Square
                # in steady state, DVE for the latency-critical first chunk),
                # cubed on DVE; B3 = 4*SQ - SP = 6*b3.
                def basis_ops(s, dve_square):
                    nc.vector.tensor_scalar(PT[:, s], A[:, s], 2.0, 0.0,
                                            AluOpType.subtract, AluOpType.min)
                    nc.vector.tensor_scalar(QT[:, s], A[:, s], 1.0, 0.0,
                                            AluOpType.subtract, AluOpType.min)
                    nc.vector.tensor_tensor(SQ[:, s], QT[:, s], QT[:, s],
                                            AluOpType.mult)
                    nc.vector.tensor_tensor(SQ[:, s], SQ[:, s], QT[:, s],
                                            AluOpType.mult)
                    if dve_square:
                        nc.vector.tensor_tensor(SP[:, s], PT[:, s], PT[:, s],
                                                AluOpType.mult)
                    else:
                        nc.scalar.activation(SP[:, s], PT[:, s], AF.Square)
                    nc.vector.tensor_tensor(SP[:, s], SP[:, s], PT[:, s],
                                            AluOpType.mult)
                    nc.vector.scalar_tensor_tensor(B3[:, s], SQ[:, s], 4.0,
                                                   SP[:, s],
                                                   AluOpType.mult,
                                                   AluOpType.subtract)

                if c == 0:
                    # quarter-split; all-DVE only for the latency-critical
                    # first quarter, ACT squares after (early DVE debt feeds
                    # the chunk-1/2 tensor stalls otherwise)
                    for q in range(4):
                        basis_ops(slice(q * 2 * BL, (q + 1) * 2 * BL), q == 0)
                elif c == 1:
                    for q in range(4):
                        basis_ops(slice(q * 2 * BL, (q + 1) * 2 * BL), False)
                else:
                    for h in range(2):
                        basis_ops(slice(h * HB, (h + 1) * HB), False)

                # weight blocks (8 spline j's + silu/base_weight), moving
                # operand.  Host layout is chunk-major: rows (c*9+j)*128+p.
                # Per-block DMAs in consumption order, alternating between the
                # sync and scalar HW queues: doubles weight bandwidth while
                # matmuls unblock block-by-block.
                JORDER = ([8, 0, 1, 2, 3, 4, 5, 6, 7] if c == 0
                          else [0, 1, 2, 3, 8, 4, 5, 6, 7])
                wt = wp.tile([128, 9 * OUT_DIM], F16, tag="wt", name=f"wt{c}")
                base = c * 9 * 128
                for j in JORDER:
                    nc.sync.dma_start(
                        out=wt[:, j * OUT_DIM:(j + 1) * OUT_DIM],
                        in_=w[base + j * 128:base + (j + 1) * 128, :])

                if c == NCH - 1:
                    # batch-major x tiles for the residual epilogue (only
                    # needed now; keeps early DMA bandwidth for weights)
                    for bt in range(NBT):
                        xbt = cp.tile([128, IN_DIM], F32, tag=f"xb{bt}")
                        nc.sync.dma_start(out=xbt[:, :],
                                            in_=xb[bt * 128:(bt + 1) * 128, :])
                        xb_tiles.append(xbt)

                def stat_of(j, bt):
                    if j < 8:
                        return B3[:, j * BL + bt * 128:j * BL + (bt + 1) * 128]
                    return sil[:, bt * 128:(bt + 1) * 128]

                def rhs_of(j, oh):
                    return wt[:, j * OUT_DIM + oh * 512:
                              j * OUT_DIM + (oh + 1) * 512]

                # consume in production order
                if c < NCH - 1:
                    for j in JORDER:
                        for bt in range(NBT):
                            for oh in range(NOH):
                                nc.tensor.matmul(
                                    psum[bt * NOH + oh][:, :],
                                    lhsT=stat_of(j, bt),
                                    rhs=rhs_of(j, oh),
                                    start=(c == 0 and j == 8),
                                    stop=False,
                                    skip_group_check=True)
                else:
                    # last chunk: bt-outer so each PSUM bank pair finishes
                    # early and its epilogue overlaps the remaining matmuls
                    for bt in range(NBT):
                        for j in JORDER:
                            for oh in range(NOH):
                                nc.tensor.matmul(
                                    psum[bt * NOH + oh][:, :],
                                    lhsT=stat_of(j, bt),
                                    rhs=rhs_of(j, oh),
                                    start=False,
                                    stop=(j == 7),
                                    skip_group_check=True)
                        for oh in range(NOH):
                            # ScalarE drains PSUM (it sits closest to PSUM),
                            # DVE adds the residual all-SBUF
                            yt = yp.tile([128, 512], F32, tag="yt",
                                         name=f"yt{bt}_{oh}")
                            nc.scalar.activation(yt[:, :],
                                                 psum[bt * NOH + oh][:, :],
                                                 AF.Copy)
                            nc.vector.scalar_tensor_tensor(
                                yt[:, :],
                                xb_tiles[bt][:, oh * 512:(oh + 1) * 512],
                                rs_t[:, :], yt[:, :],
                                AluOpType.mult, AluOpType.add)
                            nc.sync.dma_start(
                                out=y[bt * 128:(bt + 1) * 128,
                                      oh * 512:(oh + 1) * 512],
                                in_=yt[:, :])

    nc.compile()
    return nc


def _build_nc_general():
    """General-grid fallback: Cox-de Boor recursion on device."""
    nc = bacc.Bacc("TRN2", target_bir_lowering=False)

    xt = nc.dram_tensor("xt", [IN_DIM, BL], F32, kind="ExternalInput")
    w = nc.dram_tensor("w", [9 * IN_DIM, OUT_DIM], F16, kind="ExternalInput")
    gsl = nc.dram_tensor("gsl", [128, NCH * (NK - 1)], F32, kind="ExternalInput")
    gst = nc.dram_tensor("gst", [128, NCH], F32, kind="ExternalInput")
    rs = nc.dram_tensor("rs", [1, 1], F32, kind="ExternalInput")
    y = nc.dram_tensor("y", [OUT_DIM, BL], F32, kind="ExternalOutput")

    with tile.TileContext(nc) as tc:
        with (
            tc.tile_pool(name="const", bufs=1) as cp,
            tc.tile_pool(name="xres", bufs=1) as xp,
            tc.tile_pool(name="small", bufs=4) as sp,
            tc.tile_pool(name="updn", bufs=2) as bp1,
            tc.tile_pool(name="lr2", bufs=2) as bp2,
            tc.tile_pool(name="lr3", bufs=3) as bp3,
            tc.tile_pool(name="wts", bufs=12) as wp,
            tc.tile_pool(name="yout", bufs=4) as yp,
            tc.tile_pool(name="psum", bufs=1, space="PSUM") as pp,
        ):
            # ---------------- grid preparation (once) ----------------
            gslT = cp.tile([128, NK - 1, NCH], F32)
            nc.gpsimd.dma_start(out=gslT[:, :, :],
                                in_=gsl[:, :].rearrange("p (k c) -> p k c", c=NCH))
            g3 = cp.tile([128, NK, NCH], F32)
            nc.gpsimd.dma_start(out=g3[:, 0, :], in_=gst[:, :])

            # softplus(v) = relu(v) + ln(1 + exp(-|v|))
            st3 = cp.tile([128, NK - 1, NCH], F32)
            spa = cp.tile([128, NK - 1, NCH], F32)
            nc.scalar.activation(spa[:, :, :], gslT[:, :, :], AF.Abs)
            nc.scalar.activation(spa[:, :, :], spa[:, :, :], AF.Exp, scale=-1.0)
            nc.scalar.activation(spa[:, :, :], spa[:, :, :], AF.Ln, bias=1.0)
            nc.scalar.activation(st3[:, :, :], gslT[:, :, :], AF.Relu)
            nc.vector.tensor_tensor(st3[:, :, :], st3[:, :, :], spa[:, :, :],
                                    AluOpType.add)
            for k in range(1, NK):
                nc.vector.tensor_tensor(g3[:, k, :], g3[:, k - 1, :],
                                        st3[:, k - 1, :], AluOpType.add)

            def recips(d, n):
                dt = cp.tile([128, n, NCH], F32, tag=f"d{d}")
                nc.vector.tensor_tensor(dt[:, :, :], g3[:, d:NK, :],
                                        g3[:, 0:NK - d, :], AluOpType.subtract)
                nc.vector.tensor_scalar_add(dt[:, :, :], dt[:, :, :], EPS)
                r = cp.tile([128, n, NCH], F32, tag=f"r{d}")
                nc.vector.reciprocal(r[:, :, :], dt[:, :, :])
                nr = cp.tile([128, n, NCH], F32, tag=f"nr{d}")
                nc.vector.tensor_scalar_mul(nr[:, :, :], r[:, :, :], -1.0)
                return r, nr

            R1, NR1 = recips(1, NK - 1)   # [128,8,11]
            R2, NR2 = recips(2, NK - 2)   # [128,8,10]
            R3, NR3 = recips(3, NK - 3)   # [128,8,9]

            # biases for the ACT hat ops
            BU = cp.tile([128, 10, NCH], F32)   # -g[j]*R1[j]
            nc.vector.scalar_tensor_tensor(BU[:, :, :], g3[:, 0:10, :], -1.0,
                                           R1[:, 0:10, :],
                                           AluOpType.mult, AluOpType.mult)
            BD = cp.tile([128, 10, NCH], F32)   # g[j+2]*R1[j+1]
            nc.vector.tensor_tensor(BD[:, :, :], g3[:, 2:12, :],
                                    R1[:, 1:11, :], AluOpType.mult)

            # biases for the ACT degree-3 factor ops
            BL3 = cp.tile([128, 8, NCH], F32)   # -g[j]*R3[j]
            nc.vector.scalar_tensor_tensor(BL3[:, :, :], g3[:, 0:8, :], -1.0,
                                           R3[:, 0:8, :],
                                           AluOpType.mult, AluOpType.mult)
            BR3 = cp.tile([128, 8, NCH], F32)   # g[j+4]*R3[j+1]
            nc.vector.tensor_tensor(BR3[:, :, :], g3[:, 4:12, :],
                                    R3[:, 1:9, :], AluOpType.mult)

            rs_t = cp.tile([128, 1], F32)
            nc.gpsimd.dma_start(out=rs_t[:, :], in_=rs[:].to_broadcast((128, 1)))

            psum = [pp.tile([128, BL], F32, tag=f"ps{m}", name=f"ps{m}")
                    for m in range(NCH)]

            xc_tiles = []
            for c in range(NCH):
                xc = xp.tile([128, BL], F32, tag=f"xc{c}")
                nc.sync.dma_start(out=xc[:, :], in_=xt[c * 128:(c + 1) * 128, :])
                xc_tiles.append(xc)

                x16 = sp.tile([128, BL], F16, tag="x16")
                nc.vector.tensor_scalar(x16[:, :], xc[:, :], 1.0,
                                        None, AluOpType.mult)
                UP = bp1.tile([128, 10, BL], F16, tag="up")
                DN = bp1.tile([128, 10, BL], F16, tag="dn")
                for j in range(10):
                    nc.scalar.activation(UP[:, j, :], x16[:, :], AF.Relu,
                                         bias=BU[:, j, c:c+1], scale=R1[:, j, c:c+1])
                    nc.scalar.activation(DN[:, j, :], x16[:, :], AF.Relu,
                                         bias=BD[:, j, c:c+1], scale=NR1[:, j+1, c:c+1])

                L2 = bp2.tile([128, 10, BL], F16, tag="l2")
                R2t = bp2.tile([128, 9, BL], F16, tag="r2")
                L3 = bp3.tile([128, 8, BL], F16, tag="l3")
                R3t = bp3.tile([128, 8, BL], F16, tag="r3")
                for j in range(10):
                    nc.vector.tensor_scalar(L2[:, j, :], x16[:, :],
                                            g3[:, j, c:c+1], R2[:, j, c:c+1],
                                            AluOpType.subtract, AluOpType.mult)
                nc.vector.tensor_scalar(R2t[:, :, :], L2[:, 1:10, :], -1.0,
                                        1.0, AluOpType.mult, AluOpType.add)
                for j in range(3):
                    nc.vector.tensor_scalar(L3[:, j, :], x16[:, :],
                                            g3[:, j, c:c+1], R3[:, j, c:c+1],
                                            AluOpType.subtract, AluOpType.mult)
                for j in range(8):
                    if j >= 3:
                        nc.scalar.activation(L3[:, j, :], x16[:, :], AF.Identity,
                                             bias=BL3[:, j, c:c+1],
                                             scale=R3[:, j, c:c+1])
                    nc.scalar.activation(R3t[:, j, :], x16[:, :], AF.Identity,
                                         bias=BR3[:, j, c:c+1],
                                         scale=NR3[:, j+1, c:c+1])
                sil = sp.tile([128, BL], F16, tag="sil")
                nc.scalar.activation(sil[:, :], x16[:, :], AF.Silu)

                nc.vector.tensor_tensor(UP[:, :, :], UP[:, :, :], DN[:, :, :],
                                        AluOpType.min)   # b1 := UP
                nc.vector.tensor_tensor(L2[:, 0:9, :], L2[:, 0:9, :],
                                        UP[:, 0:9, :], AluOpType.mult)
                nc.vector.tensor_tensor(R2t[:, :, :], R2t[:, :, :],
                                        UP[:, 1:10, :], AluOpType.mult)
                nc.vector.tensor_tensor(L2[:, 0:9, :], L2[:, 0:9, :],
                                        R2t[:, :, :], AluOpType.add)  # b2
                nc.vector.tensor_tensor(L3[:, :, :], L3[:, :, :],
                                        L2[:, 0:8, :], AluOpType.mult)
                nc.vector.tensor_tensor(R3t[:, :, :], R3t[:, :, :],
                                        L2[:, 1:9, :], AluOpType.mult)
                nc.vector.tensor_tensor(L3[:, :, :], L3[:, :, :],
                                        R3t[:, :, :], AluOpType.add)  # b3

                wts = []
                for j in range(9):
                    kc = j * NCH + c
                    wt = wp.tile([128, OUT_DIM], F16, tag="wt", name=f"wt{c}_{j}")
                    nc.sync.dma_start(out=wt[:, :],
                                      in_=w[kc * 128:(kc + 1) * 128, :])
                    wts.append(wt)

                def rhs_of(j):
                    return L3[:, j, :] if j < 8 else sil[:, :]

                if c < NCH - 1:
                    for j in range(9):
                        for m in range(NCH):
                            nc.tensor.matmul(psum[m][:, :],
                                             lhsT=wts[j][:, m * 128:(m + 1) * 128],
                                             rhs=rhs_of(j),
                                             start=(c == 0 and j == 0),
                                             stop=False,
                                             skip_group_check=True)
                else:
                    for m in range(NCH):
                        for j in range(9):
                            nc.tensor.matmul(psum[m][:, :],
                                             lhsT=wts[j][:, m * 128:(m + 1) * 128],
                                             rhs=rhs_of(j),
                                             start=False,
                                             stop=(j == 8),
                                             skip_group_check=True)
                        yt = yp.tile([128, BL], F32, tag="yt", name=f"yt{m}")
                        nc.vector.scalar_tensor_tensor(yt[:, :],
                                                       xc_tiles[m][:, :],
                                                       rs_t[:, :], psum[m][:, :],
                                                       AluOpType.mult,
                                                       AluOpType.add)
                        nc.sync.dma_start(out=y[m * 128:(m + 1) * 128, :],
                                          in_=yt[:, :])

    nc.compile()
    return nc


_NC_CACHE = {}


def kernel(x, coeffs, base_weight, grid_steps_log, grid_start, res_scale,
           _trace=False):
    global LAST_PROFILE

    x = np.asarray(x, dtype=np.float32)
    coeffs = np.asarray(coeffs, dtype=np.float32)
    base_weight = np.asarray(base_weight, dtype=np.float32)
    grid_steps_log = np.asarray(grid_steps_log, dtype=np.float32)
    grid_start = np.asarray(grid_start, dtype=np.float32)
    res_scale = np.asarray(res_scale, dtype=np.float32)

    # ---- host-side grid analysis (float64) ----
    steps64 = np.logaddexp(0.0, grid_steps_log.astype(np.float64))  # softplus
    g0_64 = grid_start.astype(np.float64)[:, 0]
    h = float(steps64.mean())
    g0 = float(g0_64.mean())
    uniform = (np.abs(steps64 - h).max() <= 1e-6 * max(abs(h), 1e-12)
               and np.abs(g0_64 - g0).max() <= 1e-6 and h > 0)

    xT = np.ascontiguousarray(x.T)                                # [in, B]
    rs_r = res_scale.reshape(1, 1)

    # weight blocks; block j=8 is base_weight.T
    wj = coeffs.reshape(OUT_DIM, IN_DIM, 8).transpose(2, 1, 0)    # [8, in, out]
    if uniform:
        wj = wj * (1.0 / 6.0)        # fold the 1/6 of the cardinal spline
    big_w = np.concatenate([wj, base_weight.T[None]], axis=0)     # [9, in, out]
    if uniform:
        # chunk-major row order: row (c*9 + j)*128 + p  (one DMA per chunk)
        big_w = (big_w.reshape(9, NCH, 128, OUT_DIM).transpose(1, 0, 2, 3)
                 .reshape(9 * IN_DIM, OUT_DIM))
    else:
        # j-major row order: k = j*IN_DIM + i
        big_w = big_w.reshape(9 * IN_DIM, OUT_DIM)
    big_w = np.ascontiguousarray(big_w, dtype=np.float16)

    if uniform:
        key = ("uniform", round(1.0 / h, 9), round(-g0 / h, 9))
        if key not in _NC_CACHE:
            _NC_CACHE.clear()
            _NC_CACHE[key] = _build_nc_uniform(1.0 / h, -g0 / h)
        nc = _NC_CACHE[key]
        in_maps = [{
            "xt": np.ascontiguousarray(xT[:, c * BL:(c + 1) * BL]),
            "xb": np.ascontiguousarray(x[c * BL:(c + 1) * BL, :]),
            "w": big_w,
            "rs": rs_r,
        } for c in range(N_CORES)]
    else:
        key = ("general",)
        if key not in _NC_CACHE:
            _NC_CACHE.clear()
            _NC_CACHE[key] = _build_nc_general()
        nc = _NC_CACHE[key]
        gsl_r = np.ascontiguousarray(
            grid_steps_log.reshape(NCH, 128, NK - 1).transpose(1, 2, 0)
            .reshape(128, (NK - 1) * NCH))
        gst_r = np.ascontiguousarray(grid_start.reshape(NCH, 128).T)
        in_maps = [{
            "xt": np.ascontiguousarray(xT[:, c * BL:(c + 1) * BL]),
            "w": big_w,
            "gsl": gsl_r,
            "gst": gst_r,
            "rs": rs_r,
        } for c in range(N_CORES)]

    res = run_bass_kernel_spmd(nc, in_maps, core_ids=list(range(N_CORES)),
                               trace=_trace)
    LAST_PROFILE = {
        "exec_time_ns": res.exec_time_ns,
        "mean_exec_time_ns": res.mean_exec_time_ns,
        "max_exec_time_core_id": res.max_exec_time_core_id,
        "profile_json": res.profile_json,
        "instructions_and_trace": res.instructions_and_trace,
    }

    if uniform:
        out = np.concatenate([r["y"] for r in res.results], axis=0)  # [B, out]
    else:
        out = np.concatenate([r["y"].T for r in res.results], axis=0)
    return np.ascontiguousarray(out.astype(np.float32))


# revision 25
# speedup vs baseline: 1.3030x; 1.0098x over previous
"""BSpline KAN layer (grid_size=5, spline_order=3) on 8 Trainium2 NeuronCores.

Strategy (data-parallel over batch):
  - Each core gets B_local = 512 rows of x, replicated weights.
  - Layout on-chip: in-dim on partitions (8 chunks of 128), batch on free dim.

Fast path (host-detected uniform identical grid, which setup_inputs produces):
  With u = (x - g0)/h the cubic bases are the cardinal B-spline b3_j = S(u-j).
  Using the two-sided truncated-power form with m_j = 2 - |u - (j+2)|:
      b3_j = (1/6)*relu(m_j)^3 - (2/3)*relu(m_j - 1)^3
  (exact: S is symmetric about its center; for m <= 2 the remaining truncated
  terms vanish, and all values are bounded by 8 so fp16 is safe).
  Per in-chunk: 8 narrow ACT Abs ops (immediate scale/bias) + 1 ACT Square +
  7 wide DVE ops produce the full [128, 8, 512] basis stack; the 1/6 is folded
  into the spline weights on the host.  ACT ~10us/chunk, DVE ~11us/chunk, both
  below the tensor engine's ~18us/chunk matmul stream.

  - Spline contraction as matmul with k-order j-major: k = j*1024 + i, so the
    j-stacked basis tiles are directly the matmul rhs. silu(x) @ base_weight.T
    is folded in as a 9th "basis" with base_weight as its weight block.
  - All 8 PSUM banks accumulate the 8 out-chunks across the whole contraction;
    epilogue adds res_scale * x and stores y[out, batch] (host transposes).

Fallback path (general grids): Cox-de Boor recursion kernel (unchanged from
the general implementation; correct for any grid).
Precision: fp16 bases/weights, fp32 accumulation.
"""

import numpy as np

import concourse.bass as bass
from concourse import bacc
import concourse.mybir as mybir
import concourse.tile as tile
from concourse.alu_op_type import AluOpType
from concourse.bass_utils import run_bass_kernel_spmd

F32 = mybir.dt.float32
F16 = mybir.dt.float16
AF = mybir.ActivationFunctionType

IN_DIM = 1024
OUT_DIM = 1024
BATCH = 4096
N_CORES = 8
BL = BATCH // N_CORES        # 512 batch rows per core
NCH = IN_DIM // 128          # 8 in-dim chunks
NK = 12                      # knots per dim
EPS = 1e-8

LAST_PROFILE = {}


def _build_nc_uniform(inv_h, nu0):
    """Uniform-grid kernel.  u = x*inv_h + nu0 (nu0 = -g0/h);
    A_j = |u - (j+2)|, m_j = 2 - A_j,
    B3_j = 6*b3_j = relu(m_j)^3 - 4*relu(m_j-1)^3  (weights pre-divided by 6).
    Computed as: PT = min(A-2,0) = -relu(m);  QT = min(A-1,0) = -relu(m-1);
    SP = PT^2*PT = -relu(m)^3 ; SQ = -relu(m-1)^3 ; B3 = 4*SQ - SP.

    Matmuls run bases-stationary / weights-moving: lhsT = B3[in128, batch128]
    so each stationary serves both 512-wide out-halves (halves LDWEIGHTS).
    PSUM bank (bt, oh) = [128 batch, 512 out]; y is emitted [batch, out]."""
    nc = bacc.Bacc("TRN2", target_bir_lowering=False)

    xt = nc.dram_tensor("xt", [IN_DIM, BL], F16, kind="ExternalInput")
    xb = nc.dram_tensor("xb", [BL, IN_DIM], F32, kind="ExternalInput")
    w = nc.dram_tensor("w", [9 * IN_DIM, OUT_DIM], F16, kind="ExternalInput")
    rs = nc.dram_tensor("rs", [1, 1], F32, kind="ExternalInput")
    y = nc.dram_tensor("y", [BL, OUT_DIM], F32, kind="ExternalOutput")

    NBT = BL // 128              # 4 batch tiles
    NOH = OUT_DIM // 512         # 2 out halves

    with tile.TileContext(nc) as tc:
        with (
            tc.tile_pool(name="const", bufs=1) as cp,
            tc.tile_pool(name="xres", bufs=1) as xp,
            tc.tile_pool(name="sil", bufs=2) as sp,
            tc.tile_pool(name="abs", bufs=2) as apl,
            tc.tile_pool(name="cube", bufs=2) as bp,
            tc.tile_pool(name="b3", bufs=2) as b3p,
            tc.tile_pool(name="wts", bufs=3) as wp,
            tc.tile_pool(name="yout", bufs=4) as yp,
            tc.tile_pool(name="psum", bufs=1, space="PSUM") as pp,
        ):
            rs_t = cp.tile([128, 1], F32)
            nc.gpsimd.dma_start(out=rs_t[:, :], in_=rs[:].to_broadcast((128, 1)))

            # per-j ACT biases nu0 - (j+2) as [128,1] columns (no const-AP
            # registration for arbitrary floats); memset on the idle DVE so
            # they are ready before the first Abs
            bias_t = cp.tile([128, 8], F32)
            for j in range(8):
                nc.vector.memset(bias_t[:, j:j + 1], float(nu0 - (j + 2)))

            # dummy Silu: triggers the one-time load of silu_and_others (the
            # only table set needed: it contains abs/square/silu/copy) while
            # the first x tile is still streaming in
            warm = cp.tile([128, 2], F32)
            nc.vector.memset(warm[:, 0:1], 0.0)
            nc.scalar.activation(warm[:, 1:2], warm[:, 0:1], AF.Silu)

            # PSUM accumulators: bank (bt, oh) = [128 batch, 512 out]
            psum = [pp.tile([128, 512], F32, tag=f"ps{b}", name=f"ps{b}")
                    for b in range(NBT * NOH)]

            # PE clock pre-warm: dummy matmuls on zeroed tiles while the
            # first x/weight tiles stream in (results discarded by the real
            # group's start=True reset)
            dummy = cp.tile([128, 128], F16, tag="dummy")
            nc.vector.memset(dummy[:, :], 0.0)
            for k in range(20):
                nc.tensor.matmul(psum[0][:, 0:128], lhsT=dummy[:, :],
                                 rhs=dummy[:, :], start=(k == 0),
                                 stop=(k == 19), skip_group_check=True)

            HB = 4 * BL          # half-stack width (j 0-3 | 4-7)
            xb_tiles = []

            for c in range(NCH):
                xc = xp.tile([128, BL], F16, tag=f"xc{c}")
                nc.sync.dma_start(out=xc[:, :],
                                  in_=xt[c * 128:(c + 1) * 128, :])

                # flat [128, 8*BL] stacks: 1-D free dim so wide DVE ops pay the
                # SBUF inter-instruction bubble once, not per 512-row
                A = apl.tile([128, 8 * BL], F16, tag="A")
                sil = sp.tile([128, BL], F16, tag="sil")
                if c == 0:
                    # silu first: the j=8 matmul block starts the tensor
                    # engine while the Abs/cube chain is still filling
                    nc.scalar.activation(sil[:, :], xc[:, :], AF.Silu)
                for j in range(8):
                    nc.scalar.activation(A[:, j * BL:(j + 1) * BL], xc[:, :],
                                         AF.Abs, bias=bias_t[:, j:j + 1],
                                         scale=float(inv_h))
                if c > 0:
                    nc.scalar.activation(sil[:, :], xc[:, :], AF.Silu)

                PT = bp.tile([128, 8 * BL], F16, tag="PT")
                QT = bp.tile([128, 8 * BL], F16, tag="QT")
                SP = bp.tile([128, 8 * BL], F16, tag="SP")
                SQ = bp.tile([128, 8 * BL], F16, tag="SQ")
                B3 = b3p.tile([128, 8 * BL], F16, tag="B3")

                # per slice s: PT = -relu(m), QT = -relu(m-1) (4x tensor_scalar)
                # SQ = -relu(m-1)^3 via 2 DVE mults; SP = relu(m)^2 (ACT Square
                # in steady state, DVE for the latency-critical first chunk),
                # cubed on DVE; B3 = 4*SQ - SP = 6*b3.
                def basis_ops(s, dve_square):
                    nc.vector.tensor_scalar(PT[:, s], A[:, s], 2.0, 0.0,
                                            AluOpType.subtract, AluOpType.min)
                    nc.vector.tensor_scalar(QT[:, s], A[:, s], 1.0, 0.0,
                                            AluOpType.subtract, AluOpType.min)
                    nc.vector.tensor_tensor(SQ[:, s], QT[:, s], QT[:, s],
                                            AluOpType.mult)
                    nc.vector.tensor_tensor(SQ[:, s], SQ[:, s], QT[:, s],
                                            AluOpType.mult)
                    if dve_square:
                        nc.vector.tensor_tensor(SP[:, s], PT[:, s], PT[:, s],
                                                AluOpType.mult)
                    else:
                        nc.scalar.activation(SP[:, s], PT[:, s], AF.Square)
                    nc.vector.tensor_tensor(SP[:, s], SP[:, s], PT[:, s],
                                            AluOpType.mult)
                    nc.vector.scalar_tensor_tensor(B3[:, s], SQ[:, s], 4.0,
                                                   SP[:, s],
                                                   AluOpType.mult,
                                                   AluOpType.subtract)

                if c == 0:
                    # quarter-split; all-DVE only for the latency-critical
                    # first quarter, ACT squares after (early DVE debt feeds
                    # the chunk-1/2 tensor stalls otherwise)
                    for q in range(4):
                        basis_ops(slice(q * 2 * BL, (q + 1) * 2 * BL), q == 0)
                elif c == 1:
                    for q in range(4):
                        basis_ops(slice(q * 2 * BL, (q + 1) * 2 * BL), False)
                else:
                    for h in range(2):
                        basis_ops(slice(h * HB, (h + 1) * HB), False)

                # weight blocks (8 spline j's + silu/base_weight), moving
                # operand.  Host layout is chunk-major: rows (c*9+j)*128+p.
                # Per-block DMAs in consumption order, alternating between the
                # sync and scalar HW queues: doubles weight bandwidth while
                # matmuls unblock block-by-block.
                JORDER = ([8, 0, 1, 2, 3, 4, 5, 6, 7] if c == 0
                          else [0, 1, 2, 3, 8, 4, 5, 6, 7])
                wt = wp.tile([128, 9 * OUT_DIM], F16, tag="wt", name=f"wt{c}")
                base = c * 9 * 128
                for j in JORDER:
                    nc.sync.dma_start(
                        out=wt[:, j * OUT_DIM:(j + 1) * OUT_DIM],
                        in_=w[base + j * 128:base + (j + 1) * 128, :])

                if c == NCH - 1:
                    # batch-major x tiles for the residual epilogue (only
                    # needed now; keeps early DMA bandwidth for weights)
                    for bt in range(NBT):
                        xbt = cp.tile([128, IN_DIM], F32, tag=f"xb{bt}")
                        nc.sync.dma_start(out=xbt[:, :],
                                            in_=xb[bt * 128:(bt + 1) * 128, :])
                        xb_tiles.append(xbt)

                def stat_of(j, bt):
                    if j < 8:
                        return B3[:, j * BL + bt * 128:j * BL + (bt + 1) * 128]
                    return sil[:, bt * 128:(bt + 1) * 128]

                def rhs_of(j, oh):
                    return wt[:, j * OUT_DIM + oh * 512:
                              j * OUT_DIM + (oh + 1) * 512]

                # consume in production order
                if c < NCH - 1:
                    for j in JORDER:
                        for bt in range(NBT):
                            for oh in range(NOH):
                                nc.tensor.matmul(
                                    psum[bt * NOH + oh][:, :],
                                    lhsT=stat_of(j, bt),
                                    rhs=rhs_of(j, oh),
                                    start=(c == 0 and j == 8),
                                    stop=False,
                                    skip_group_check=True)
                else:
                    # last chunk: bt-outer so each PSUM bank pair finishes
                    # early and its epilogue overlaps the remaining matmuls
                    for bt in range(NBT):
                        for j in JORDER:
                            for oh in range(NOH):
                                nc.tensor.matmul(
                                    psum[bt * NOH + oh][:, :],
                                    lhsT=stat_of(j, bt),
                                    rhs=rhs_of(j, oh),
                                    start=False,
                                    stop=(j == 7),
                                    skip_group_check=True)
                        for oh in range(NOH):
                            # ScalarE drains PSUM (it sits closest to PSUM),
                            # DVE adds the residual all-SBUF
                            yt = yp.tile([128, 512], F32, tag="yt",
                                         name=f"yt{bt}_{oh}")
                            nc.scalar.activation(yt[:, :],
                                                 psum[bt * NOH + oh][:, :],
                                                 AF.Copy)
                            nc.vector.scalar_tensor_tensor(
                                yt[:, :],
                                xb_tiles[bt][:, oh * 512:(oh + 1) * 512],
                                rs_t[:, :], yt[:, :],
                                AluOpType.mult, AluOpType.add)
                            nc.sync.dma_start(
                                out=y[bt * 128:(bt + 1) * 128,
                                      oh * 512:(oh + 1) * 512],
                                in_=yt[:, :])

    nc.compile()
    return nc


def _build_nc_general():
    """General-grid fallback: Cox-de Boor recursion on device."""
    nc = bacc.Bacc("TRN2", target_bir_lowering=False)

    xt = nc.dram_tensor("xt", [IN_DIM, BL], F32, kind="ExternalInput")
    w = nc.dram_tensor("w", [9 * IN_DIM, OUT_DIM], F16, kind="ExternalInput")
    gsl = nc.dram_tensor("gsl", [128, NCH * (NK - 1)], F32, kind="ExternalInput")
    gst = nc.dram_tensor("gst", [128, NCH], F32, kind="ExternalInput")
    rs = nc.dram_tensor("rs", [1, 1], F32, kind="ExternalInput")
    y = nc.dram_tensor("y", [OUT_DIM, BL], F32, kind="ExternalOutput")

    with tile.TileContext(nc) as tc:
        with (
            tc.tile_pool(name="const", bufs=1) as cp,
            tc.tile_pool(name="xres", bufs=1) as xp,
            tc.tile_pool(name="small", bufs=4) as sp,
            tc.tile_pool(name="updn", bufs=2) as bp1,
            tc.tile_pool(name="lr2", bufs=2) as bp2,
            tc.tile_pool(name="lr3", bufs=3) as bp3,
            tc.tile_pool(name="wts", bufs=12) as wp,
            tc.tile_pool(name="yout", bufs=4) as yp,
            tc.tile_pool(name="psum", bufs=1, space="PSUM") as pp,
        ):
            # ---------------- grid preparation (once) ----------------
            gslT = cp.tile([128, NK - 1, NCH], F32)
            nc.gpsimd.dma_start(out=gslT[:, :, :],
                                in_=gsl[:, :].rearrange("p (k c) -> p k c", c=NCH))
            g3 = cp.tile([128, NK, NCH], F32)
            nc.gpsimd.dma_start(out=g3[:, 0, :], in_=gst[:, :])

            # softplus(v) = relu(v) + ln(1 + exp(-|v|))
            st3 = cp.tile([128, NK - 1, NCH], F32)
            spa = cp.tile([128, NK - 1, NCH], F32)
            nc.scalar.activation(spa[:, :, :], gslT[:, :, :], AF.Abs)
            nc.scalar.activation(spa[:, :, :], spa[:, :, :], AF.Exp, scale=-1.0)
            nc.scalar.activation(spa[:, :, :], spa[:, :, :], AF.Ln, bias=1.0)
            nc.scalar.activation(st3[:, :, :], gslT[:, :, :], AF.Relu)
            nc.vector.tensor_tensor(st3[:, :, :], st3[:, :, :], spa[:, :, :],
                                    AluOpType.add)
            for k in range(1, NK):
                nc.vector.tensor_tensor(g3[:, k, :], g3[:, k - 1, :],
                                        st3[:, k - 1, :], AluOpType.add)

            def recips(d, n):
                dt = cp.tile([128, n, NCH], F32, tag=f"d{d}")
                nc.vector.tensor_tensor(dt[:, :, :], g3[:, d:NK, :],
                                        g3[:, 0:NK - d, :], AluOpType.subtract)
                nc.vector.tensor_scalar_add(dt[:, :, :], dt[:, :, :], EPS)
                r = cp.tile([128, n, NCH], F32, tag=f"r{d}")
                nc.vector.reciprocal(r[:, :, :], dt[:, :, :])
                nr = cp.tile([128, n, NCH], F32, tag=f"nr{d}")
                nc.vector.tensor_scalar_mul(nr[:, :, :], r[:, :, :], -1.0)
                return r, nr

            R1, NR1 = recips(1, NK - 1)   # [128,8,11]
            R2, NR2 = recips(2, NK - 2)   # [128,8,10]
            R3, NR3 = recips(3, NK - 3)   # [128,8,9]

            # biases for the ACT hat ops
            BU = cp.tile([128, 10, NCH], F32)   # -g[j]*R1[j]
            nc.vector.scalar_tensor_tensor(BU[:, :, :], g3[:, 0:10, :], -1.0,
                                           R1[:, 0:10, :],
                                           AluOpType.mult, AluOpType.mult)
            BD = cp.tile([128, 10, NCH], F32)   # g[j+2]*R1[j+1]
            nc.vector.tensor_tensor(BD[:, :, :], g3[:, 2:12, :],
                                    R1[:, 1:11, :], AluOpType.mult)

            # biases for the ACT degree-3 factor ops
            BL3 = cp.tile([128, 8, NCH], F32)   # -g[j]*R3[j]
            nc.vector.scalar_tensor_tensor(BL3[:, :, :], g3[:, 0:8, :], -1.0,
                                           R3[:, 0:8, :],
                                           AluOpType.mult, AluOpType.mult)
            BR3 = cp.tile([128, 8, NCH], F32)   # g[j+4]*R3[j+1]
            nc.vector.tensor_tensor(BR3[:, :, :], g3[:, 4:12, :],
                                    R3[:, 1:9, :], AluOpType.mult)

            rs_t = cp.tile([128, 1], F32)
            nc.gpsimd.dma_start(out=rs_t[:, :], in_=rs[:].to_broadcast((128, 1)))

            psum = [pp.tile([128, BL], F32, tag=f"ps{m}", name=f"ps{m}")
                    for m in range(NCH)]

            xc_tiles = []
            for c in range(NCH):
                xc = xp.tile([128, BL], F32, tag=f"xc{c}")
                nc.sync.dma_start(out=xc[:, :], in_=xt[c * 128:(c + 1) * 128, :])
                xc_tiles.append(xc)

                x16 = sp.tile([128, BL], F16, tag="x16")
                nc.vector.tensor_scalar(x16[:, :], xc[:, :], 1.0,
                                        None, AluOpType.mult)
                UP = bp1.tile([128, 10, BL], F16, tag="up")
                DN = bp1.tile([128, 10, BL], F16, tag="dn")
                for j in range(10):
                    nc.scalar.activation(UP[:, j, :], x16[:, :], AF.Relu,
                                         bias=BU[:, j, c:c+1], scale=R1[:, j, c:c+1])
                    nc.scalar.activation(DN[:, j, :], x16[:, :], AF.Relu,
                                         bias=BD[:, j, c:c+1], scale=NR1[:, j+1, c:c+1])

                L2 = bp2.tile([128, 10, BL], F16, tag="l2")
                R2t = bp2.tile([128, 9, BL], F16, tag="r2")
                L3 = bp3.tile([128, 8, BL], F16, tag="l3")
                R3t = bp3.tile([128, 8, BL], F16, tag="r3")
                for j in range(10):
                    nc.vector.tensor_scalar(L2[:, j, :], x16[:, :],
                                            g3[:, j, c:c+1], R2[:, j, c:c+1],
                                            AluOpType.subtract, AluOpType.mult)
                nc.vector.tensor_scalar(R2t[:, :, :], L2[:, 1:10, :], -1.0,
                                        1.0, AluOpType.mult, AluOpType.add)
                for j in range(3):
                    nc.vector.tensor_scalar(L3[:, j, :], x16[:, :],
                                            g3[:, j, c:c+1], R3[:, j, c:c+1],
                                            AluOpType.subtract, AluOpType.mult)
                for j in range(8):
                    if j >= 3:
                        nc.scalar.activation(L3[:, j, :], x16[:, :], AF.Identity,
                                             bias=BL3[:, j, c:c+1],
                                             scale=R3[:, j, c:c+1])
                    nc.scalar.activation(R3t[:, j, :], x16[:, :], AF.Identity,
                                         bias=BR3[:, j, c:c+1],
                                         scale=NR3[:, j+1, c:c+1])
                sil = sp.tile([128, BL], F16, tag="sil")
                nc.scalar.activation(sil[:, :], x16[:, :], AF.Silu)

                nc.vector.tensor_tensor(UP[:, :, :], UP[:, :, :], DN[:, :, :],
                                        AluOpType.min)   # b1 := UP
                nc.vector.tensor_tensor(L2[:, 0:9, :], L2[:, 0:9, :],
                                        UP[:, 0:9, :], AluOpType.mult)
                nc.vector.tensor_tensor(R2t[:, :, :], R2t[:, :, :],
                                        UP[:, 1:10, :], AluOpType.mult)
                nc.vector.tensor_tensor(L2[:, 0:9, :], L2[:, 0:9, :],
                                        R2t[:, :, :], AluOpType.add)  # b2
                nc.vector.tensor_tensor(L3[:, :, :], L3[:, :, :],
                                        L2[:, 0:8, :], AluOpType.mult)
                nc.vector.tensor_tensor(R3t[:, :, :], R3t[:, :, :],
                                        L2[:, 1:9, :], AluOpType.mult)
                nc.vector.tensor_tensor(L3[:, :, :], L3[:, :, :],
                                        R3t[:, :, :], AluOpType.add)  # b3

                wts = []
                for j in range(9):
                    kc = j * NCH + c
                    wt = wp.tile([128, OUT_DIM], F16, tag="wt", name=f"wt{c}_{j}")
                    nc.sync.dma_start(out=wt[:, :],
                                      in_=w[kc * 128:(kc + 1) * 128, :])
                    wts.append(wt)

                def rhs_of(j):
                    return L3[:, j, :] if j < 8 else sil[:, :]

                if c < NCH - 1:
                    for j in range(9):
                        for m in range(NCH):
                            nc.tensor.matmul(psum[m][:, :],
                                             lhsT=wts[j][:, m * 128:(m + 1) * 128],
                                             rhs=rhs_of(j),
                                             start=(c == 0 and j == 0),
                                             stop=False,
                                             skip_group_check=True)
                else:
                    for m in range(NCH):
                        for j in range(9):
                            nc.tensor.matmul(psum[m][:, :],
                                             lhsT=wts[j][:, m * 128:(m + 1) * 128],
                                             rhs=rhs_of(j),
                                             start=False,
                                             stop=(j == 8),
                                             skip_group_check=True)
                        yt = yp.tile([128, BL], F32, tag="yt", name=f"yt{m}")
                        nc.vector.scalar_tensor_tensor(yt[:, :],
                                                       xc_tiles[m][:, :],
                                                       rs_t[:, :], psum[m][:, :],
                                                       AluOpType.mult,
                                                       AluOpType.add)
                        nc.sync.dma_start(out=y[m * 128:(m + 1) * 128, :],
                                          in_=yt[:, :])

    nc.compile()
    return nc


_NC_CACHE = {}


def kernel(x, coeffs, base_weight, grid_steps_log, grid_start, res_scale,
           _trace=False):
    global LAST_PROFILE

    x = np.asarray(x, dtype=np.float32)
    coeffs = np.asarray(coeffs, dtype=np.float32)
    base_weight = np.asarray(base_weight, dtype=np.float32)
    grid_steps_log = np.asarray(grid_steps_log, dtype=np.float32)
    grid_start = np.asarray(grid_start, dtype=np.float32)
    res_scale = np.asarray(res_scale, dtype=np.float32)

    # ---- host-side grid analysis (float64) ----
    steps64 = np.logaddexp(0.0, grid_steps_log.astype(np.float64))  # softplus
    g0_64 = grid_start.astype(np.float64)[:, 0]
    h = float(steps64.mean())
    g0 = float(g0_64.mean())
    uniform = (np.abs(steps64 - h).max() <= 1e-6 * max(abs(h), 1e-12)
               and np.abs(g0_64 - g0).max() <= 1e-6 and h > 0)

    xT = np.ascontiguousarray(x.T)                                # [in, B]
    rs_r = res_scale.reshape(1, 1)

    # weight blocks; block j=8 is base_weight.T
    wj = coeffs.reshape(OUT_DIM, IN_DIM, 8).transpose(2, 1, 0)    # [8, in, out]
    if uniform:
        wj = wj * (1.0 / 6.0)        # fold the 1/6 of the cardinal spline
    big_w = np.concatenate([wj, base_weight.T[None]], axis=0)     # [9, in, out]
    if uniform:
        # chunk-major row order: row (c*9 + j)*128 + p  (one DMA per chunk)
        big_w = (big_w.reshape(9, NCH, 128, OUT_DIM).transpose(1, 0, 2, 3)
                 .reshape(9 * IN_DIM, OUT_DIM))
    else:
        # j-major row order: k = j*IN_DIM + i
        big_w = big_w.reshape(9 * IN_DIM, OUT_DIM)
    big_w = np.ascontiguousarray(big_w, dtype=np.float16)

    if uniform:
        key = ("uniform", round(1.0 / h, 9), round(-g0 / h, 9))
        if key not in _NC_CACHE:
            _NC_CACHE.clear()
            _NC_CACHE[key] = _build_nc_uniform(1.0 / h, -g0 / h)
        nc = _NC_CACHE[key]
        xT16 = xT.astype(np.float16)
        in_maps = [{
            "xt": np.ascontiguousarray(xT16[:, c * BL:(c + 1) * BL]),
            "xb": np.ascontiguousarray(x[c * BL:(c + 1) * BL, :]),
            "w": big_w,
            "rs": rs_r,
        } for c in range(N_CORES)]
    else:
        key = ("general",)
        if key not in _NC_CACHE:
            _NC_CACHE.clear()
            _NC_CACHE[key] = _build_nc_general()
        nc = _NC_CACHE[key]
        gsl_r = np.ascontiguousarray(
            grid_steps_log.reshape(NCH, 128, NK - 1).transpose(1, 2, 0)
            .reshape(128, (NK - 1) * NCH))
        gst_r = np.ascontiguousarray(grid_start.reshape(NCH, 128).T)
        in_maps = [{
            "xt": np.ascontiguousarray(xT[:, c * BL:(c + 1) * BL]),
            "w": big_w,
            "gsl": gsl_r,
            "gst": gst_r,
            "rs": rs_r,
        } for c in range(N_CORES)]

    res = run_bass_kernel_spmd(nc, in_maps, core_ids=list(range(N_CORES)),
                               trace=_trace)
    LAST_PROFILE = {
        "exec_time_ns": res.exec_time_ns,
        "mean_exec_time_ns": res.mean_exec_time_ns,
        "max_exec_time_core_id": res.max_exec_time_core_id,
        "profile_json": res.profile_json,
        "instructions_and_trace": res.instructions_and_trace,
    }

    if uniform:
        out = np.concatenate([r["y"] for r in res.results], axis=0)  # [B, out]
    else:
        out = np.concatenate([r["y"].T for r in res.results], axis=0)
    return np.ascontiguousarray(out.astype(np.float32))
